# revision 2
# baseline (speedup 1.0000x reference)
"""Trainium2 Bass kernel v2 for a 2-layer GCN with data-aware attention gate.

Math (per reference):
    src,dst = edges + self-loops; deg = bincount(dst); dinv = rsqrt(deg)
    norm = dinv[src]*dinv[dst]
    h1 = relu(segsum(norm * (x@W1)[src], dst) + b1)
    h2 = relu(segsum(norm * (h1@W2)[src], dst) + b2)
    out = h2 * sigmoid(h2@attn_w + attn_b)

v2 design (8 NeuronCores, dst-sharded, 64-slot windows):
  - Tables are pure bf16, node-major, pair-packed: gather element = two
    consecutive slots' rows = 256B (dma_gather minimum).  Both layers share
    ONE edge plan: idx = s_pos>>1 (int16-safe), parity h = s_pos&1 selects
    the element half fed to the PE.  Layer-1 rows are 64 bf16 feats; layer-2
    rows are [32 feats | 32 zeros].
  - Transposed aggregation: matmul(lhsT=gathered-chunk[128e x used],
    rhs=S[128e x 64 slots]) accumulates feat-major [used, 64] PSUM per
    window; no per-window PE transposes.
  - S one-hot matrices built ONE wide is_equal per gather-group via
    stride-0 broadcast APs (dval col vs iota row).
  - Self-loops are excluded from edge lists; handled as one PE matmul per
    window: lhsT = own shard rows (SBUF-resident), rhs = diag(dinv_w).
  - All per-partition scalings ride scalar-engine ACT scale/bias; no
    tensor_scalar with AP operands anywhere.
"""

import sys

import numpy as np

_CONC = "/opt/trn_rl_repo"
if _CONC not in sys.path:
    sys.path.insert(0, _CONC)

# ---------------------------------------------------------------------------
# configuration
# ---------------------------------------------------------------------------


class Cfg:
    def __init__(self, N=50000, DIN=128, DH=64, DOUT=32, NC=8, W64=98, WPG=14):
        self.N, self.DIN, self.DH, self.DOUT = N, DIN, DH, DOUT
        self.NC, self.W64, self.WPG = NC, W64, WPG
        assert W64 % WPG == 0 and W64 % 2 == 0
        self.G = W64 // WPG
        self.NB = W64 // 2              # 128-node blocks per core
        self.NPC = W64 * 64             # slots per core
        self.TOT = NC * self.NPC
        self.PAIRS = self.TOT // 2
        assert self.PAIRS <= 32768      # int16 gather indices
        assert self.N <= self.TOT


FULL = Cfg()

# ---------------------------------------------------------------------------
# host-side graph prep (structure only)
# ---------------------------------------------------------------------------


def _assign_slots(load, cfg):
    """LPT-deal nodes into NC*W64 bins of <=64 slots, balancing `load`."""
    import heapq

    nbins = cfg.NC * cfg.W64
    order = np.argsort(-load, kind="stable")
    heap = [(0, b) for b in range(nbins)]
    heapq.heapify(heap)
    count = np.zeros(nbins, np.int64)
    pos = np.empty(cfg.N, np.int64)
    for n in order:
        l, b = heapq.heappop(heap)
        pos[n] = b * 64 + count[b]
        count[b] += 1
        if count[b] < 64:
            heapq.heappush(heap, (l + int(load[n]), b))
    return pos


def prep(x, edge_index, cfg):
    N, NC, W64, WPG, G = cfg.N, cfg.NC, cfg.W64, cfg.WPG, cfg.G
    NPC, DIN = cfg.NPC, cfg.DIN

    src = edge_index[0].astype(np.int64)
    dst = edge_index[1].astype(np.int64)
    deg_in = np.bincount(dst, minlength=N).astype(np.int64)
    deg = (deg_in + 1).astype(np.float32)          # + self-loop
    dinv = (1.0 / np.sqrt(np.maximum(deg, 1e-12))).astype(np.float32)

    pos = _assign_slots(deg_in, cfg)

    # per-slot tables
    node_of = np.full(cfg.TOT, -1, np.int64)
    node_of[pos] = np.arange(N)
    dinv_slot = np.ones(cfg.TOT, np.float32)
    dinv_slot[pos] = dinv

    # edge records (no self-loops)
    s_pos = pos[src]
    d_pos = pos[dst]
    c_e = d_pos // NPC
    w_e = (d_pos % NPC) // 64
    dval_e = (d_pos % 64 + 2).astype(np.float32)
    h_e = s_pos & 1
    gidx_e = s_pos >> 1

    # bucket edges by (core, window, half)
    key_all = (c_e * W64 + w_e) * 2 + h_e
    order_e = np.argsort(key_all, kind="stable")
    ks = key_all[order_e]
    bounds = np.searchsorted(ks, np.arange(NC * W64 * 2 + 1))
    buckets = {}
    for key in range(NC * W64 * 2):
        lo, hi = bounds[key], bounds[key + 1]
        if hi > lo:
            buckets[key] = order_e[lo:hi]

    # SPMD-uniform chunk targets
    tgt = np.zeros((W64, 2), np.int64)
    for w in range(W64):
        for h in range(2):
            mx = max(len(buckets.get((c * W64 + w) * 2 + h, ()))
                     for c in range(NC))
            tgt[w, h] = int(np.ceil(max(mx, 1) / 128) * 128)

    seglen = np.zeros((G, 2), np.int64)
    for g in range(G):
        for h in range(2):
            seglen[g, h] = tgt[g * WPG:(g + 1) * WPG, h].sum()

    idx_cols = int(sum(int(seglen[g, h]) // 16
                       for g in range(G) for h in range(2)))
    chunk_tot = int(sum(int(seglen[g, h]) // 128
                        for g in range(G) for h in range(2)))
    ioff, coff = {}, {}
    io = co = 0
    for g in range(G):
        for h in range(2):
            ioff[(g, h)] = io
            coff[(g, h)] = co
            io += int(seglen[g, h]) // 16
            co += int(seglen[g, h]) // 128
    wcol = np.zeros((W64, 2), np.int64)
    for g in range(G):
        for h in range(2):
            c0 = coff[(g, h)]
            for wl in range(WPG):
                w = g * WPG + wl
                wcol[w, h] = c0
                c0 += int(tgt[w, h]) // 128

    import ml_dtypes
    bf16 = ml_dtypes.bfloat16
    ix_all = np.zeros((NC, 128, idx_cols), np.int16)
    dvl_all = np.full((NC, 128, chunk_tot), -1.0, np.float32)
    for c in range(NC):
        for g in range(G):
            for h in range(2):
                n = int(seglen[g, h])
                gi = np.zeros(n, np.int64)
                dv = np.full(n, -1.0, np.float32)
                p = 0
                for wl in range(WPG):
                    w = g * WPG + wl
                    es = buckets.get((c * W64 + w) * 2 + h, ())
                    ne = len(es)
                    gi[p:p + ne] = gidx_e[es]
                    dv[p:p + ne] = dval_e[es]
                    p += int(tgt[w, h])
                wrapped = gi.reshape(n // 16, 16).T.astype(np.int16)
                ix_all[c, :, ioff[(g, h)]:ioff[(g, h)] + n // 16] = np.tile(
                    wrapped, (8, 1))
                dvl_all[c, :, coff[(g, h)]:coff[(g, h)] + n // 128] = (
                    dv.reshape(n // 128, 128).T)

    # per-core dense tables
    X_all = np.zeros((cfg.TOT, DIN), np.float32)
    X_all[pos] = np.asarray(x, np.float32)
    xT_sh = np.zeros((NC, DIN, NPC), bf16)
    dv128 = np.zeros((NC, 128, cfg.NB), np.float32)
    dv64 = np.zeros((NC, 64, W64), np.float32)
    dvrep = np.zeros((NC, 64, NPC), np.float32)
    dvdiag = np.zeros((NC, 128, NPC), bf16)
    j = np.arange(NPC)
    for c in range(NC):
        sl = slice(c * NPC, (c + 1) * NPC)
        xT_sh[c] = X_all[sl].T.astype(bf16)
        ds = dinv_slot[sl]
        dv128[c] = ds.reshape(cfg.NB, 128).T
        dv64[c] = ds.reshape(W64, 64).T
        dvrep[c] = np.tile(ds, (64, 1))
        # self-loop selection band: identity (rows already carry dinv[s];
        # the flush applies dinv[d]), duplicated in both partition halves
        # so it can pair with lhsT at base partition 0 or 64
        dd = np.zeros((128, NPC), np.float32)
        dd[j % 64, j] = 1.0
        dd[64 + (j % 64), j] = 1.0
        dvdiag[c] = dd.astype(bf16)

    plan = dict(tgt=tgt, seglen=seglen, ioff=ioff, coff=coff, wcol=wcol,
                idx_cols=idx_cols, chunk_tot=chunk_tot)
    host = dict(xT_sh=xT_sh, dv128=dv128, dv64=dv64, dvrep=dvrep,
                dvdiag=dvdiag, ix_all=ix_all,
                dvl_all=dvl_all.astype(bf16), pos=pos)
    return plan, host


# ---------------------------------------------------------------------------
# device kernel
# ---------------------------------------------------------------------------


def build(cfg, plan, dbg=False):
    import concourse.bass as bass  # noqa: F401
    import concourse.mybir as mybir
    import concourse.tile as tile
    from concourse import bacc

    NC, W64, WPG, G, NB = cfg.NC, cfg.W64, cfg.WPG, cfg.G, cfg.NB
    NPC, TOT, PAIRS = cfg.NPC, cfg.TOT, cfg.PAIRS
    DH, DOUT = cfg.DH, cfg.DOUT
    f32 = mybir.dt.float32
    bf16 = mybir.dt.bfloat16
    AF = mybir.ActivationFunctionType
    tgt, seglen = plan["tgt"], plan["seglen"]
    ioff, coff, wcol = plan["ioff"], plan["coff"], plan["wcol"]

    nc = bacc.Bacc(
        "TRN2", target_bir_lowering=False, debug=False,
        num_devices=NC, num_swdge_queues=4,
    )

    xT_d = nc.dram_tensor("xT", [128, NPC], bf16, kind="ExternalInput")
    w1_d = nc.dram_tensor("w1", [128, DH], bf16, kind="ExternalInput")
    w2_d = nc.dram_tensor("w2", [DH, DOUT], bf16, kind="ExternalInput")
    dv128_d = nc.dram_tensor("dv128", [128, NB], f32, kind="ExternalInput")
    dv64_d = nc.dram_tensor("dv64", [64, W64], f32, kind="ExternalInput")
    dvrep_d = nc.dram_tensor("dvrep", [64, NPC], f32, kind="ExternalInput")
    dvdiag_d = nc.dram_tensor("dvdiag", [128, NPC], bf16,
                              kind="ExternalInput")
    b1_d = nc.dram_tensor("b1c", [DH, 1], f32, kind="ExternalInput")
    b2_d = nc.dram_tensor("b2c", [DOUT, 1], f32, kind="ExternalInput")
    idaw_d = nc.dram_tensor("idaw", [DOUT, DOUT + 1], bf16,
                            kind="ExternalInput")
    ab_d = nc.dram_tensor("abc", [64, 1], f32, kind="ExternalInput")
    gi_d = nc.dram_tensor("gi64", [128, 64], bf16, kind="ExternalInput")
    ix_d = nc.dram_tensor("ix", [128, plan["idx_cols"]], mybir.dt.int16,
                          kind="ExternalInput")
    dvl_d = nc.dram_tensor("dvl", [128, plan["chunk_tot"]], bf16,
                           kind="ExternalInput")
    out_d = nc.dram_tensor("out_sh", [NPC, DOUT], f32, kind="ExternalOutput")
    if dbg:
        t1dump_d = nc.dram_tensor("t1dump", [NPC, DH], bf16,
                                  kind="ExternalOutput")
        t2dump_d = nc.dram_tensor("t2dump", [NPC, DH], bf16,
                                  kind="ExternalOutput")
        h1dump_d = nc.dram_tensor("h1dump", [64, cfg.W64 * 64], f32,
                                  kind="ExternalOutput")

    rg = [list(range(NC))]
    qctr = [0]

    with tile.TileContext(nc) as tc:
        with tc.tile_pool(name="const", bufs=1) as cpool:
            def load(dram, shape, dt=f32):
                t = cpool.tile(shape, dt, tag=dram.name, name=dram.name + "_s")
                nc.sync.dma_start(t[:], dram.ap())
                return t

            w1_s = load(w1_d, [128, DH], bf16)
            w2_s = load(w2_d, [DH, DOUT], bf16)
            dv128_s = load(dv128_d, [128, NB])
            dv64_s = load(dv64_d, [64, W64])
            dvrep_s = load(dvrep_d, [64, NPC])
            dvdiag_s = load(dvdiag_d, [128, NPC], bf16)
            b1_s = load(b1_d, [DH, 1])
            b2_s = load(b2_d, [DOUT, 1])
            idaw_s = load(idaw_d, [DOUT, DOUT + 1], bf16)
            ab_s = load(ab_d, [64, 1])
            gi_s = load(gi_d, [128, 64], bf16)
            ix_s = load(ix_d, [128, plan["idx_cols"]], mybir.dt.int16)
            dvl_s = load(dvl_d, [128, plan["chunk_tot"]], bf16)

            t1sb = cpool.tile([128, NB * 64], bf16, tag="t1sb", name="t1sb")
            t2sb = cpool.tile([64, W64 * 64], bf16, tag="t2sb", name="t2sb")
            h2at = cpool.tile([64, W64 * 33], f32, tag="h2at", name="h2at")
            nc.vector.memset(t2sb[:], 0.0)

            with tc.tile_pool(name="dram", bufs=1, space="DRAM") as dpool:
                t1_shard = dpool.tile([NPC, DH], bf16, tag="t1s", name="t1s")
                t1_full = dpool.tile([TOT, DH], bf16, tag="t1f", name="t1f",
                                     addr_space="Shared")
                t2_shard = dpool.tile([NPC, DH], bf16, tag="t2s", name="t2s")
                t2_full = dpool.tile([TOT, DH], bf16, tag="t2f", name="t2f",
                                     addr_space="Shared")

                # ---- phase 1: t1 = dinv .* (x @ W1), node-major bf16
                with (
                    tc.tile_pool(name="ph1", bufs=1) as ph1,
                    tc.tile_pool(name="ph1ps", bufs=4, space="PSUM") as pps,
                ):
                    xts = ph1.tile([128, NPC], bf16, tag="xts", name="xts")
                    nc.sync.dma_start(xts[:], xT_d.ap())
                    for b in range(NB):
                        ps = pps.tile([128, DH], f32, tag="p1", name="p1")
                        nc.tensor.matmul(ps[:],
                                         lhsT=xts[:, b * 128:(b + 1) * 128],
                                         rhs=w1_s[:], start=True, stop=True)
                        nc.scalar.activation(
                            t1sb[:, b * 64:(b + 1) * 64], ps[:],
                            func=AF.Copy, scale=dv128_s[:, b:b + 1])
                        nc.sync.dma_start(
                            t1_shard[b * 128:(b + 1) * 128, :],
                            t1sb[:, b * 64:(b + 1) * 64])

                # ---- AllGather layer-1 table
                nc.gpsimd.collective_compute(
                    "AllGather", mybir.AluOpType.bypass, replica_groups=rg,
                    ins=[t1_shard[:]], outs=[t1_full[:]],
                )

                # ---- shared aggregation loop
                def aggregate(full, used, sl_lhs, flush_fn):
                    fv = full.rearrange("(a b) d -> a (b d)", b=2)
                    with (
                        tc.tile_pool(name="gp", bufs=2) as gp,
                        tc.tile_pool(name="sp", bufs=2) as sp,
                        tc.tile_pool(name="aps", bufs=4, space="PSUM") as aps,
                        tc.tile_pool(name="fsb", bufs=3) as fsb,
                        tc.tile_pool(name="fps", bufs=2, space="PSUM") as fps,
                    ):
                        for g in range(G):
                            gts, Ss = {}, {}
                            for h in range(2):
                                n = int(seglen[g, h])
                                nch = n // 128
                                gt = gp.tile([128, n], bf16, tag=f"g{h}",
                                             name=f"gt{h}")
                                io = ioff[(g, h)]
                                n1 = (nch // 2) * 128
                                for (o0, nn) in ((0, n1), (n1, n - n1)):
                                    if nn == 0:
                                        continue
                                    nc.gpsimd.dma_gather(
                                        out_ap=gt[:, o0:o0 + nn].rearrange(
                                            "p (c d) -> p c d", d=128),
                                        in_ap=fv,
                                        idxs_ap=ix_s[:, io + o0 // 16:
                                                     io + (o0 + nn) // 16],
                                        num_idxs=nn, num_idxs_reg=nn,
                                        elem_size=128, elem_step=128,
                                        queue_num=qctr[0] % 4,
                                        single_packet=False,
                                    )
                                    qctr[0] += 1
                                c0 = coff[(g, h)]
                                S = sp.tile([128, nch * 64], bf16,
                                            tag=f"S{h}", name=f"S{h}")
                                nc.vector.tensor_tensor(
                                    out=S[:].rearrange(
                                        "p (c j) -> p c j", j=64),
                                    in0=dvl_s[:, c0:c0 + nch].unsqueeze(2)
                                    .broadcast_to((128, nch, 64)),
                                    in1=gi_s[:].unsqueeze(1)
                                    .broadcast_to((128, nch, 64)),
                                    op=mybir.AluOpType.is_equal,
                                )
                                gts[h], Ss[h] = gt, S
                            for wl in range(WPG):
                                w = g * WPG + wl
                                ps = aps.tile([used, 64], f32, tag="agg",
                                              name="agg")
                                lhsT_sl, rhs_sl = sl_lhs(w)
                                nc.tensor.matmul(
                                    ps[:], lhsT=lhsT_sl, rhs=rhs_sl,
                                    start=True, stop=False)
                                chunks = (
                                    [(0, k)
                                     for k in range(int(tgt[w, 0]) // 128)]
                                    + [(1, k)
                                       for k in range(int(tgt[w, 1]) // 128)])
                                for j, (h, k) in enumerate(chunks):
                                    kk = int(wcol[w, h] - coff[(g, h)]) + k
                                    base = kk * 128 + h * 64
                                    nc.tensor.matmul(
                                        ps[:],
                                        lhsT=gts[h][:, base:base + used],
                                        rhs=Ss[h][:, kk * 64:(kk + 1) * 64],
                                        start=False,
                                        stop=(j == len(chunks) - 1))
                                flush_fn(w, ps, fsb, fps)

                # ---- layer-1 flush
                def flush1(w, ps, fsb, fps):
                    a = fsb.tile([64, 64], f32, tag="a", name="a")
                    nc.vector.tensor_tensor(
                        out=a[:], in0=ps[:],
                        in1=dvrep_s[:, w * 64:(w + 1) * 64],
                        op=mybir.AluOpType.mult)
                    hT = fsb.tile([64, 64], bf16, tag="hT", name="hT")
                    nc.scalar.activation(hT[:], a[:], func=AF.Relu,
                                         bias=b1_s[:, 0:1])
                    if dbg:
                        nc.sync.dma_start(
                            h1dump_d.ap()[:, w * 64:(w + 1) * 64], a[:])
                    t2ps = fps.tile([64, DOUT], f32, tag="t2ps", name="t2ps")
                    nc.tensor.matmul(t2ps[:], lhsT=hT[:], rhs=w2_s[:],
                                     start=True, stop=True)
                    nc.scalar.activation(
                        t2sb[:, w * 64:w * 64 + DOUT], t2ps[:],
                        func=AF.Copy, scale=dv64_s[:, w:w + 1])
                    nc.sync.dma_start(
                        t2_shard[w * 64:(w + 1) * 64, :],
                        t2sb[:, w * 64:(w + 1) * 64])

                def sl1(w):
                    po = (w & 1) * 64
                    b = w >> 1
                    return (t1sb[po:po + 64, b * 64:(b + 1) * 64],
                            dvdiag_s[po:po + 64, w * 64:(w + 1) * 64])

                aggregate(t1_full[:], DH, sl1, flush1)

                # ---- AllGather layer-2 table
                nc.gpsimd.collective_compute(
                    "AllGather", mybir.AluOpType.bypass, replica_groups=rg,
                    ins=[t2_shard[:]], outs=[t2_full[:]],
                )

                # ---- layer-2 flush
                def flush2(w, ps, fsb, fps):
                    a2 = fsb.tile([DOUT, 64], f32, tag="a2", name="a2")
                    nc.vector.tensor_tensor(
                        out=a2[:], in0=ps[:],
                        in1=dvrep_s[:DOUT, w * 64:(w + 1) * 64],
                        op=mybir.AluOpType.mult)
                    h2T = fsb.tile([DOUT, 64], bf16, tag="h2T", name="h2T")
                    nc.scalar.activation(h2T[:], a2[:], func=AF.Relu,
                                         bias=b2_s[:, 0:1])
                    gps = fps.tile([64, DOUT + 1], f32, tag="gps", name="gps")
                    nc.tensor.matmul(gps[:], lhsT=h2T[:], rhs=idaw_s[:],
                                     start=True, stop=True)
                    nc.scalar.activation(
                        h2at[:, w * 33:(w + 1) * 33], gps[:], func=AF.Copy)

                def sl2(w):
                    return (t2sb[:, w * 64:w * 64 + DOUT],
                            dvdiag_s[0:64, w * 64:(w + 1) * 64])

                aggregate(t2_full[:], DOUT, sl2, flush2)

                if dbg:
                    nc.sync.dma_start(t1dump_d.ap(), t1_shard[:])
                    nc.sync.dma_start(t2dump_d.ap(), t2_shard[:])

                # ---- attention gate tail
                with tc.tile_pool(name="tail", bufs=1) as tp:
                    atall = tp.tile([64, W64], f32, tag="atall", name="atall")
                    nc.scalar.activation(
                        atall[:],
                        h2at[:].rearrange("p (w q) -> p w q", q=33)[:, :, 32],
                        func=AF.Sigmoid, bias=ab_s[:, 0:1])
                    oall = tp.tile([64, W64 * DOUT], f32, tag="oall",
                                   name="oall")
                    nc.vector.tensor_tensor(
                        out=oall[:].rearrange("p (w f) -> p w f", f=DOUT),
                        in0=h2at[:].rearrange(
                            "p (w q) -> p w q", q=33)[:, :, 0:DOUT],
                        in1=atall[:].unsqueeze(2)
                        .broadcast_to((64, W64, DOUT)),
                        op=mybir.AluOpType.mult)
                    nc.sync.dma_start(
                        out_d.ap().rearrange("(w p) f -> p w f", p=64),
                        oall[:].rearrange("p (w f) -> p w f", f=DOUT))

    nc.compile()
    return nc


# ---------------------------------------------------------------------------
# entry point
# ---------------------------------------------------------------------------


def _make_in_maps(cfg, host, W1, b1, W2, b2, attn_w, attn_b):
    import ml_dtypes
    bf16 = ml_dtypes.bfloat16
    giota = np.tile(np.arange(2, 66, dtype=np.float32),
                    (128, 1)).astype(bf16)
    idaw = np.concatenate(
        [np.eye(cfg.DOUT, dtype=np.float32),
         np.asarray(attn_w, np.float32).reshape(cfg.DOUT, 1)],
        axis=1).astype(bf16)
    in_maps = []
    for c in range(cfg.NC):
        in_maps.append({
            "xT": host["xT_sh"][c],
            "w1": np.asarray(W1, np.float32).astype(bf16),
            "w2": np.asarray(W2, np.float32).astype(bf16),
            "dv128": host["dv128"][c],
            "dv64": host["dv64"][c],
            "dvrep": host["dvrep"][c],
            "dvdiag": host["dvdiag"][c],
            "b1c": np.asarray(b1, np.float32).reshape(cfg.DH, 1),
            "b2c": np.asarray(b2, np.float32).reshape(cfg.DOUT, 1),
            "idaw": idaw,
            "abc": np.full((64, 1),
                           np.asarray(attn_b, np.float32).reshape(-1)[0],
                           np.float32),
            "gi64": giota,
            "ix": host["ix_all"][c],
            "dvl": host["dvl_all"][c],
        })
    return in_maps


def run(x, edge_index, W1, b1, W2, b2, attn_w, attn_b, cfg=None,
        backend="hw", trace=False):
    cfg = cfg or FULL
    plan, host = prep(x, edge_index, cfg)
    nc = build(cfg, plan)
    in_maps = _make_in_maps(cfg, host, W1, b1, W2, b2, attn_w, attn_b)

    if backend == "sim":
        from concourse.bass_interp import MultiCoreSim
        sim = MultiCoreSim(nc, num_cores=cfg.NC, trace=False)
        for c, core in enumerate(sim.cores.values()):
            for name, arr in in_maps[c].items():
                core.tensor(name)[:] = arr
        sim.simulate()
        outs = [core.tensor("out_sh").copy() for core in sim.cores.values()]
        exec_ns = None
    else:
        from concourse import bass_utils
        from concourse.bass_interp import get_hw_module
        old = nc.m
        nc.m = get_hw_module(nc.m)
        try:
            res = bass_utils.run_bass_kernel_spmd(
                nc, in_maps, core_ids=list(range(cfg.NC)), trace=trace)
        finally:
            nc.m = old
        outs = [res.results[c]["out_sh"] for c in range(cfg.NC)]
        exec_ns = res.exec_time_ns

    full = np.concatenate(outs, axis=0)   # [TOT, DOUT] in slot order
    out = full[host["pos"]]               # unpermute -> [N, DOUT]
    return np.ascontiguousarray(out), exec_ns


def kernel(x, edge_index, W1, b1, W2, b2, attn_w, attn_b):
    out, _ = run(x, edge_index, W1, b1, W2, b2, attn_w, attn_b,
                 cfg=FULL, backend="hw", trace=False)
    return out


# revision 4
# speedup vs baseline: 1.0140x; 1.0140x over previous
"""Trainium2 Bass kernel v2 for a 2-layer GCN with data-aware attention gate.

Math (per reference):
    src,dst = edges + self-loops; deg = bincount(dst); dinv = rsqrt(deg)
    norm = dinv[src]*dinv[dst]
    h1 = relu(segsum(norm * (x@W1)[src], dst) + b1)
    h2 = relu(segsum(norm * (h1@W2)[src], dst) + b2)
    out = h2 * sigmoid(h2@attn_w + attn_b)

v2 design (8 NeuronCores, dst-sharded, 64-slot windows):
  - Tables are pure bf16, node-major, pair-packed: gather element = two
    consecutive slots' rows = 256B (dma_gather minimum).  Both layers share
    ONE edge plan: idx = s_pos>>1 (int16-safe), parity h = s_pos&1 selects
    the element half fed to the PE.  Layer-1 rows are 64 bf16 feats; layer-2
    rows are [32 feats | 32 zeros].
  - Transposed aggregation: matmul(lhsT=gathered-chunk[128e x used],
    rhs=S[128e x 64 slots]) accumulates feat-major [used, 64] PSUM per
    window; no per-window PE transposes.
  - S one-hot matrices built ONE wide is_equal per gather-group via
    stride-0 broadcast APs (dval col vs iota row).
  - Self-loops are excluded from edge lists; handled as one PE matmul per
    window: lhsT = own shard rows (SBUF-resident), rhs = diag(dinv_w).
  - All per-partition scalings ride scalar-engine ACT scale/bias; no
    tensor_scalar with AP operands anywhere.
"""

import sys

import numpy as np

_CONC = "/opt/trn_rl_repo"
if _CONC not in sys.path:
    sys.path.insert(0, _CONC)

# ---------------------------------------------------------------------------
# configuration
# ---------------------------------------------------------------------------


class Cfg:
    def __init__(self, N=50000, DIN=128, DH=64, DOUT=32, NC=8, W64=98, WPG=14):
        self.N, self.DIN, self.DH, self.DOUT = N, DIN, DH, DOUT
        self.NC, self.W64, self.WPG = NC, W64, WPG
        assert W64 % WPG == 0 and W64 % 2 == 0
        self.G = W64 // WPG
        self.NB = W64 // 2              # 128-node blocks per core
        self.NPC = W64 * 64             # slots per core
        self.TOT = NC * self.NPC
        self.PAIRS = self.TOT // 2
        assert self.PAIRS <= 32768      # int16 gather indices
        assert self.N <= self.TOT


FULL = Cfg()

# ---------------------------------------------------------------------------
# host-side graph prep (structure only)
# ---------------------------------------------------------------------------


def _assign_slots(load, cfg):
    """LPT-deal nodes into NC*W64 bins of <=64 slots, balancing `load`."""
    import heapq

    nbins = cfg.NC * cfg.W64
    order = np.argsort(-load, kind="stable")
    heap = [(0, b) for b in range(nbins)]
    heapq.heapify(heap)
    count = np.zeros(nbins, np.int64)
    pos = np.empty(cfg.N, np.int64)
    for n in order:
        l, b = heapq.heappop(heap)
        pos[n] = b * 64 + count[b]
        count[b] += 1
        if count[b] < 64:
            heapq.heappush(heap, (l + int(load[n]), b))
    return pos


def prep(x, edge_index, cfg):
    N, NC, W64, WPG, G = cfg.N, cfg.NC, cfg.W64, cfg.WPG, cfg.G
    NPC, DIN = cfg.NPC, cfg.DIN

    src = edge_index[0].astype(np.int64)
    dst = edge_index[1].astype(np.int64)
    deg_in = np.bincount(dst, minlength=N).astype(np.int64)
    deg = (deg_in + 1).astype(np.float32)          # + self-loop
    dinv = (1.0 / np.sqrt(np.maximum(deg, 1e-12))).astype(np.float32)

    pos = _assign_slots(deg_in, cfg)

    # per-slot tables
    node_of = np.full(cfg.TOT, -1, np.int64)
    node_of[pos] = np.arange(N)
    dinv_slot = np.ones(cfg.TOT, np.float32)
    dinv_slot[pos] = dinv

    # edge records (no self-loops)
    s_pos = pos[src]
    d_pos = pos[dst]
    c_e = d_pos // NPC
    w_e = (d_pos % NPC) // 64
    dval_e = (d_pos % 64 + 2).astype(np.float32)
    h_e = s_pos & 1
    gidx_e = s_pos >> 1

    # bucket edges by (core, window, half)
    key_all = (c_e * W64 + w_e) * 2 + h_e
    order_e = np.argsort(key_all, kind="stable")
    ks = key_all[order_e]
    bounds = np.searchsorted(ks, np.arange(NC * W64 * 2 + 1))
    buckets = {}
    for key in range(NC * W64 * 2):
        lo, hi = bounds[key], bounds[key + 1]
        if hi > lo:
            buckets[key] = order_e[lo:hi]

    # SPMD-uniform chunk targets
    tgt = np.zeros((W64, 2), np.int64)
    for w in range(W64):
        for h in range(2):
            mx = max(len(buckets.get((c * W64 + w) * 2 + h, ()))
                     for c in range(NC))
            tgt[w, h] = int(np.ceil(max(mx, 1) / 128) * 128)

    seglen = np.zeros((G, 2), np.int64)
    for g in range(G):
        for h in range(2):
            seglen[g, h] = tgt[g * WPG:(g + 1) * WPG, h].sum()

    idx_cols = int(sum(int(seglen[g, h]) // 16
                       for g in range(G) for h in range(2)))
    chunk_tot = int(sum(int(seglen[g, h]) // 128
                        for g in range(G) for h in range(2)))
    ioff, coff = {}, {}
    io = co = 0
    for g in range(G):
        for h in range(2):
            ioff[(g, h)] = io
            coff[(g, h)] = co
            io += int(seglen[g, h]) // 16
            co += int(seglen[g, h]) // 128
    wcol = np.zeros((W64, 2), np.int64)
    for g in range(G):
        for h in range(2):
            c0 = coff[(g, h)]
            for wl in range(WPG):
                w = g * WPG + wl
                wcol[w, h] = c0
                c0 += int(tgt[w, h]) // 128

    import ml_dtypes
    bf16 = ml_dtypes.bfloat16
    ix_all = np.zeros((NC, 128, idx_cols), np.int16)
    dvl_all = np.full((NC, 128, chunk_tot), -1.0, np.float32)
    for c in range(NC):
        for g in range(G):
            for h in range(2):
                n = int(seglen[g, h])
                gi = np.zeros(n, np.int64)
                dv = np.full(n, -1.0, np.float32)
                p = 0
                for wl in range(WPG):
                    w = g * WPG + wl
                    es = buckets.get((c * W64 + w) * 2 + h, ())
                    ne = len(es)
                    if ne:
                        # sort by gather idx for HBM row-buffer locality
                        es = np.asarray(es)[np.argsort(gidx_e[es],
                                                       kind="stable")]
                    gi[p:p + ne] = gidx_e[es]
                    dv[p:p + ne] = dval_e[es]
                    p += int(tgt[w, h])
                wrapped = gi.reshape(n // 16, 16).T.astype(np.int16)
                ix_all[c, :, ioff[(g, h)]:ioff[(g, h)] + n // 16] = np.tile(
                    wrapped, (8, 1))
                dvl_all[c, :, coff[(g, h)]:coff[(g, h)] + n // 128] = (
                    dv.reshape(n // 128, 128).T)

    # per-core dense tables
    X_all = np.zeros((cfg.TOT, DIN), np.float32)
    X_all[pos] = np.asarray(x, np.float32)
    xT_sh = np.zeros((NC, DIN, NPC), bf16)
    dv128 = np.zeros((NC, 128, cfg.NB), np.float32)
    dv64 = np.zeros((NC, 64, W64), np.float32)
    dvrep = np.zeros((NC, 64, NPC), np.float32)
    dvdiag = np.zeros((NC, 128, NPC), bf16)
    j = np.arange(NPC)
    for c in range(NC):
        sl = slice(c * NPC, (c + 1) * NPC)
        xT_sh[c] = X_all[sl].T.astype(bf16)
        ds = dinv_slot[sl]
        dv128[c] = ds.reshape(cfg.NB, 128).T
        dv64[c] = ds.reshape(W64, 64).T
        dvrep[c] = np.tile(ds, (64, 1))
        # self-loop selection band: identity (rows already carry dinv[s];
        # the flush applies dinv[d]), duplicated in both partition halves
        # so it can pair with lhsT at base partition 0 or 64
        dd = np.zeros((128, NPC), np.float32)
        dd[j % 64, j] = 1.0
        dd[64 + (j % 64), j] = 1.0
        dvdiag[c] = dd.astype(bf16)

    plan = dict(tgt=tgt, seglen=seglen, ioff=ioff, coff=coff, wcol=wcol,
                idx_cols=idx_cols, chunk_tot=chunk_tot)
    host = dict(xT_sh=xT_sh, dv128=dv128, dv64=dv64, dvrep=dvrep,
                dvdiag=dvdiag, ix_all=ix_all,
                dvl_all=dvl_all.astype(bf16), pos=pos)
    return plan, host


# ---------------------------------------------------------------------------
# device kernel
# ---------------------------------------------------------------------------


def build(cfg, plan, dbg=False):
    import concourse.bass as bass  # noqa: F401
    import concourse.mybir as mybir
    import concourse.tile as tile
    from concourse import bacc

    NC, W64, WPG, G, NB = cfg.NC, cfg.W64, cfg.WPG, cfg.G, cfg.NB
    NPC, TOT, PAIRS = cfg.NPC, cfg.TOT, cfg.PAIRS
    DH, DOUT = cfg.DH, cfg.DOUT
    f32 = mybir.dt.float32
    bf16 = mybir.dt.bfloat16
    AF = mybir.ActivationFunctionType
    tgt, seglen = plan["tgt"], plan["seglen"]
    ioff, coff, wcol = plan["ioff"], plan["coff"], plan["wcol"]

    nc = bacc.Bacc(
        "TRN2", target_bir_lowering=False, debug=False,
        num_devices=NC, num_swdge_queues=4,
    )

    xT_d = nc.dram_tensor("xT", [128, NPC], bf16, kind="ExternalInput")
    w1_d = nc.dram_tensor("w1", [128, DH], bf16, kind="ExternalInput")
    w2_d = nc.dram_tensor("w2", [DH, DOUT], bf16, kind="ExternalInput")
    dv128_d = nc.dram_tensor("dv128", [128, NB], f32, kind="ExternalInput")
    dv64_d = nc.dram_tensor("dv64", [64, W64], f32, kind="ExternalInput")
    dvrep_d = nc.dram_tensor("dvrep", [64, NPC], f32, kind="ExternalInput")
    dvdiag_d = nc.dram_tensor("dvdiag", [128, NPC], bf16,
                              kind="ExternalInput")
    b1_d = nc.dram_tensor("b1c", [DH, 1], f32, kind="ExternalInput")
    b2_d = nc.dram_tensor("b2c", [DOUT, 1], f32, kind="ExternalInput")
    idaw_d = nc.dram_tensor("idaw", [DOUT, DOUT + 1], bf16,
                            kind="ExternalInput")
    ab_d = nc.dram_tensor("abc", [64, 1], f32, kind="ExternalInput")
    gi_d = nc.dram_tensor("gi64", [128, 64], bf16, kind="ExternalInput")
    ix_d = nc.dram_tensor("ix", [128, plan["idx_cols"]], mybir.dt.int16,
                          kind="ExternalInput")
    dvl_d = nc.dram_tensor("dvl", [128, plan["chunk_tot"]], bf16,
                           kind="ExternalInput")
    out_d = nc.dram_tensor("out_sh", [NPC, DOUT], f32, kind="ExternalOutput")
    if dbg:
        t1dump_d = nc.dram_tensor("t1dump", [NPC, DH], bf16,
                                  kind="ExternalOutput")
        t2dump_d = nc.dram_tensor("t2dump", [NPC, DH], bf16,
                                  kind="ExternalOutput")
        h1dump_d = nc.dram_tensor("h1dump", [64, cfg.W64 * 64], f32,
                                  kind="ExternalOutput")

    rg = [list(range(NC))]
    qctr = [0]

    with tile.TileContext(nc) as tc:
        with tc.tile_pool(name="const", bufs=1) as cpool:
            def load(dram, shape, dt=f32):
                t = cpool.tile(shape, dt, tag=dram.name, name=dram.name + "_s")
                nc.sync.dma_start(t[:], dram.ap())
                return t

            w1_s = load(w1_d, [128, DH], bf16)
            w2_s = load(w2_d, [DH, DOUT], bf16)
            dv128_s = load(dv128_d, [128, NB])
            dv64_s = load(dv64_d, [64, W64])
            dvrep_s = load(dvrep_d, [64, NPC])
            dvdiag_s = load(dvdiag_d, [128, NPC], bf16)
            b1_s = load(b1_d, [DH, 1])
            b2_s = load(b2_d, [DOUT, 1])
            idaw_s = load(idaw_d, [DOUT, DOUT + 1], bf16)
            ab_s = load(ab_d, [64, 1])
            gi_s = load(gi_d, [128, 64], bf16)
            ix_s = load(ix_d, [128, plan["idx_cols"]], mybir.dt.int16)
            dvl_s = load(dvl_d, [128, plan["chunk_tot"]], bf16)

            t1sb = cpool.tile([128, NB * 64], bf16, tag="t1sb", name="t1sb")
            t2sb = cpool.tile([64, W64 * 64], bf16, tag="t2sb", name="t2sb")
            h2at = cpool.tile([64, W64 * 33], f32, tag="h2at", name="h2at")
            nc.vector.memset(t2sb[:], 0.0)

            with tc.tile_pool(name="dram", bufs=1, space="DRAM") as dpool:
                t1_shard = dpool.tile([NPC, DH], bf16, tag="t1s", name="t1s")
                t1_full = dpool.tile([TOT, DH], bf16, tag="t1f", name="t1f",
                                     addr_space="Shared")
                t2_shard = dpool.tile([NPC, DH], bf16, tag="t2s", name="t2s")
                t2_full = dpool.tile([TOT, DH], bf16, tag="t2f", name="t2f",
                                     addr_space="Shared")

                # ---- phase 1: t1 = dinv .* (x @ W1), node-major bf16
                with (
                    tc.tile_pool(name="ph1", bufs=1) as ph1,
                    tc.tile_pool(name="ph1ps", bufs=4, space="PSUM") as pps,
                ):
                    xts = ph1.tile([128, NPC], bf16, tag="xts", name="xts")
                    nc.sync.dma_start(xts[:], xT_d.ap())
                    for b in range(NB):
                        ps = pps.tile([128, DH], f32, tag="p1", name="p1")
                        nc.tensor.matmul(ps[:],
                                         lhsT=xts[:, b * 128:(b + 1) * 128],
                                         rhs=w1_s[:], start=True, stop=True)
                        nc.scalar.activation(
                            t1sb[:, b * 64:(b + 1) * 64], ps[:],
                            func=AF.Copy, scale=dv128_s[:, b:b + 1])
                        nc.sync.dma_start(
                            t1_shard[b * 128:(b + 1) * 128, :],
                            t1sb[:, b * 64:(b + 1) * 64])

                # ---- AllGather layer-1 table
                nc.gpsimd.collective_compute(
                    "AllGather", mybir.AluOpType.bypass, replica_groups=rg,
                    ins=[t1_shard[:]], outs=[t1_full[:]],
                )

                # ---- shared aggregation loop
                def aggregate(full, used, sl_lhs, flush_fn):
                    fv = full.rearrange("(a b) d -> a (b d)", b=2)
                    with (
                        tc.tile_pool(name="gp", bufs=2) as gp,
                        tc.tile_pool(name="sp", bufs=2) as sp,
                        tc.tile_pool(name="aps", bufs=4, space="PSUM") as aps,
                        tc.tile_pool(name="fsb", bufs=3) as fsb,
                        tc.tile_pool(name="fps", bufs=2, space="PSUM") as fps,
                    ):
                        for g in range(G):
                            gts, Ss = {}, {}
                            for h in range(2):
                                n = int(seglen[g, h])
                                nch = n // 128
                                gt = gp.tile([128, n], bf16, tag=f"g{h}",
                                             name=f"gt{h}")
                                io = ioff[(g, h)]
                                n1 = (nch // 2) * 128
                                for (o0, nn) in ((0, n1), (n1, n - n1)):
                                    if nn == 0:
                                        continue
                                    nc.gpsimd.dma_gather(
                                        out_ap=gt[:, o0:o0 + nn].rearrange(
                                            "p (c d) -> p c d", d=128),
                                        in_ap=fv,
                                        idxs_ap=ix_s[:, io + o0 // 16:
                                                     io + (o0 + nn) // 16],
                                        num_idxs=nn, num_idxs_reg=nn,
                                        elem_size=128, elem_step=128,
                                        queue_num=qctr[0] % 4,
                                        single_packet=False,
                                    )
                                    qctr[0] += 1
                                c0 = coff[(g, h)]
                                S = sp.tile([128, nch * 64], bf16,
                                            tag=f"S{h}", name=f"S{h}")
                                nc.vector.tensor_tensor(
                                    out=S[:].rearrange(
                                        "p (c j) -> p c j", j=64),
                                    in0=dvl_s[:, c0:c0 + nch].unsqueeze(2)
                                    .broadcast_to((128, nch, 64)),
                                    in1=gi_s[:].unsqueeze(1)
                                    .broadcast_to((128, nch, 64)),
                                    op=mybir.AluOpType.is_equal,
                                )
                                gts[h], Ss[h] = gt, S
                            for wl in range(WPG):
                                w = g * WPG + wl
                                ps = aps.tile([used, 64], f32, tag="agg",
                                              name="agg")
                                lhsT_sl, rhs_sl = sl_lhs(w)
                                nc.tensor.matmul(
                                    ps[:], lhsT=lhsT_sl, rhs=rhs_sl,
                                    start=True, stop=False)
                                chunks = (
                                    [(0, k)
                                     for k in range(int(tgt[w, 0]) // 128)]
                                    + [(1, k)
                                       for k in range(int(tgt[w, 1]) // 128)])
                                for j, (h, k) in enumerate(chunks):
                                    kk = int(wcol[w, h] - coff[(g, h)]) + k
                                    base = kk * 128 + h * 64
                                    nc.tensor.matmul(
                                        ps[:],
                                        lhsT=gts[h][:, base:base + used],
                                        rhs=Ss[h][:, kk * 64:(kk + 1) * 64],
                                        start=False,
                                        stop=(j == len(chunks) - 1))
                                flush_fn(w, ps, fsb, fps)

                # ---- layer-1 flush
                def flush1(w, ps, fsb, fps):
                    a = fsb.tile([64, 64], f32, tag="a", name="a")
                    nc.vector.tensor_tensor(
                        out=a[:], in0=ps[:],
                        in1=dvrep_s[:, w * 64:(w + 1) * 64],
                        op=mybir.AluOpType.mult)
                    hT = fsb.tile([64, 64], bf16, tag="hT", name="hT")
                    nc.scalar.activation(hT[:], a[:], func=AF.Relu,
                                         bias=b1_s[:, 0:1])
                    if dbg:
                        nc.sync.dma_start(
                            h1dump_d.ap()[:, w * 64:(w + 1) * 64], a[:])
                    t2ps = fps.tile([64, DOUT], f32, tag="t2ps", name="t2ps")
                    nc.tensor.matmul(t2ps[:], lhsT=hT[:], rhs=w2_s[:],
                                     start=True, stop=True)
                    nc.scalar.activation(
                        t2sb[:, w * 64:w * 64 + DOUT], t2ps[:],
                        func=AF.Copy, scale=dv64_s[:, w:w + 1])
                    nc.sync.dma_start(
                        t2_shard[w * 64:(w + 1) * 64, :],
                        t2sb[:, w * 64:(w + 1) * 64])

                def sl1(w):
                    po = (w & 1) * 64
                    b = w >> 1
                    return (t1sb[po:po + 64, b * 64:(b + 1) * 64],
                            dvdiag_s[po:po + 64, w * 64:(w + 1) * 64])

                aggregate(t1_full[:], DH, sl1, flush1)

                # ---- AllGather layer-2 table
                nc.gpsimd.collective_compute(
                    "AllGather", mybir.AluOpType.bypass, replica_groups=rg,
                    ins=[t2_shard[:]], outs=[t2_full[:]],
                )

                # ---- layer-2 flush
                def flush2(w, ps, fsb, fps):
                    a2 = fsb.tile([DOUT, 64], f32, tag="a2", name="a2")
                    nc.vector.tensor_tensor(
                        out=a2[:], in0=ps[:],
                        in1=dvrep_s[:DOUT, w * 64:(w + 1) * 64],
                        op=mybir.AluOpType.mult)
                    h2T = fsb.tile([DOUT, 64], bf16, tag="h2T", name="h2T")
                    nc.scalar.activation(h2T[:], a2[:], func=AF.Relu,
                                         bias=b2_s[:, 0:1])
                    gps = fps.tile([64, DOUT + 1], f32, tag="gps", name="gps")
                    nc.tensor.matmul(gps[:], lhsT=h2T[:], rhs=idaw_s[:],
                                     start=True, stop=True)
                    nc.scalar.activation(
                        h2at[:, w * 33:(w + 1) * 33], gps[:], func=AF.Copy)

                def sl2(w):
                    return (t2sb[:, w * 64:w * 64 + DOUT],
                            dvdiag_s[0:64, w * 64:(w + 1) * 64])

                aggregate(t2_full[:], DOUT, sl2, flush2)

                if dbg:
                    nc.sync.dma_start(t1dump_d.ap(), t1_shard[:])
                    nc.sync.dma_start(t2dump_d.ap(), t2_shard[:])

                # ---- attention gate tail
                with tc.tile_pool(name="tail", bufs=1) as tp:
                    atall = tp.tile([64, W64], f32, tag="atall", name="atall")
                    nc.scalar.activation(
                        atall[:],
                        h2at[:].rearrange("p (w q) -> p w q", q=33)[:, :, 32],
                        func=AF.Sigmoid, bias=ab_s[:, 0:1])
                    oall = tp.tile([64, W64 * DOUT], f32, tag="oall",
                                   name="oall")
                    nc.vector.tensor_tensor(
                        out=oall[:].rearrange("p (w f) -> p w f", f=DOUT),
                        in0=h2at[:].rearrange(
                            "p (w q) -> p w q", q=33)[:, :, 0:DOUT],
                        in1=atall[:].unsqueeze(2)
                        .broadcast_to((64, W64, DOUT)),
                        op=mybir.AluOpType.mult)
                    nc.sync.dma_start(
                        out_d.ap().rearrange("(w p) f -> p w f", p=64),
                        oall[:].rearrange("p (w f) -> p w f", f=DOUT))

    nc.compile()
    return nc


# ---------------------------------------------------------------------------
# entry point
# ---------------------------------------------------------------------------


def _make_in_maps(cfg, host, W1, b1, W2, b2, attn_w, attn_b):
    import ml_dtypes
    bf16 = ml_dtypes.bfloat16
    giota = np.tile(np.arange(2, 66, dtype=np.float32),
                    (128, 1)).astype(bf16)
    idaw = np.concatenate(
        [np.eye(cfg.DOUT, dtype=np.float32),
         np.asarray(attn_w, np.float32).reshape(cfg.DOUT, 1)],
        axis=1).astype(bf16)
    in_maps = []
    for c in range(cfg.NC):
        in_maps.append({
            "xT": host["xT_sh"][c],
            "w1": np.asarray(W1, np.float32).astype(bf16),
            "w2": np.asarray(W2, np.float32).astype(bf16),
            "dv128": host["dv128"][c],
            "dv64": host["dv64"][c],
            "dvrep": host["dvrep"][c],
            "dvdiag": host["dvdiag"][c],
            "b1c": np.asarray(b1, np.float32).reshape(cfg.DH, 1),
            "b2c": np.asarray(b2, np.float32).reshape(cfg.DOUT, 1),
            "idaw": idaw,
            "abc": np.full((64, 1),
                           np.asarray(attn_b, np.float32).reshape(-1)[0],
                           np.float32),
            "gi64": giota,
            "ix": host["ix_all"][c],
            "dvl": host["dvl_all"][c],
        })
    return in_maps


def run(x, edge_index, W1, b1, W2, b2, attn_w, attn_b, cfg=None,
        backend="hw", trace=False):
    cfg = cfg or FULL
    plan, host = prep(x, edge_index, cfg)
    nc = build(cfg, plan)
    in_maps = _make_in_maps(cfg, host, W1, b1, W2, b2, attn_w, attn_b)

    if backend == "sim":
        from concourse.bass_interp import MultiCoreSim
        sim = MultiCoreSim(nc, num_cores=cfg.NC, trace=False)
        for c, core in enumerate(sim.cores.values()):
            for name, arr in in_maps[c].items():
                core.tensor(name)[:] = arr
        sim.simulate()
        outs = [core.tensor("out_sh").copy() for core in sim.cores.values()]
        exec_ns = None
    else:
        from concourse import bass_utils
        from concourse.bass_interp import get_hw_module
        old = nc.m
        nc.m = get_hw_module(nc.m)
        try:
            res = bass_utils.run_bass_kernel_spmd(
                nc, in_maps, core_ids=list(range(cfg.NC)), trace=trace)
        finally:
            nc.m = old
        outs = [res.results[c]["out_sh"] for c in range(cfg.NC)]
        exec_ns = res.exec_time_ns

    full = np.concatenate(outs, axis=0)   # [TOT, DOUT] in slot order
    out = full[host["pos"]]               # unpermute -> [N, DOUT]
    return np.ascontiguousarray(out), exec_ns


def kernel(x, edge_index, W1, b1, W2, b2, attn_w, attn_b):
    out, _ = run(x, edge_index, W1, b1, W2, b2, attn_w, attn_b,
                 cfg=FULL, backend="hw", trace=False)
    return out


# revision 5
# speedup vs baseline: 1.6144x; 1.5921x over previous
"""Trainium2 Bass kernel v3 for a 2-layer GCN with data-aware attention gate.

Math (per reference):
    src,dst = edges + self-loops; deg = bincount(dst); dinv = rsqrt(deg)
    norm = dinv[src]*dinv[dst]
    h1 = relu(segsum(norm * (x@W1)[src], dst) + b1)
    h2 = relu(segsum(norm * (h1@W2)[src], dst) + b2)
    out = h2 * sigmoid(h2@attn_w + attn_b)

v3 design (8 NeuronCores, dst-sharded, 128-slot windows):
  - Tables are pure bf16, node-major, pair-packed: gather element = two
    consecutive slots' rows = 256B.  Both layers share ONE edge plan:
    idx = s_pos>>1 (int16-safe), parity h = s_pos&1 selects the element
    half.  Layer-1 rows are 64 bf16 feats; layer-2 rows [32 feats|32 z].
  - Gathers are split into <=2048-element pieces across the 4 SWDGE
    queues so the 128-entry descriptor FIFO never starves the SDMA
    engines (measured 2.5x drain-rate difference).
  - Transposed aggregation: matmul(lhsT=gathered-chunk[128e x used],
    rhs=S[128e x 128 slots]) accumulates feat-major [used, 128] PSUM per
    window; no per-window PE transposes.
  - S one-hot matrices built ONE wide is_equal per gather-group via
    stride-0 broadcast APs (dval col vs iota row).
  - Self-loops excluded from edge lists; handled as one PE matmul per
    window: lhsT = own shard rows (SBUF-resident), rhs = identity.
  - All per-partition scalings ride scalar-engine ACT scale/bias.
"""

import sys

import numpy as np

_CONC = "/opt/trn_rl_repo"
if _CONC not in sys.path:
    sys.path.insert(0, _CONC)

# ---------------------------------------------------------------------------
# configuration
# ---------------------------------------------------------------------------


class Cfg:
    def __init__(self, N=50000, DIN=128, DH=64, DOUT=32, NC=8, WPC=49, WPG=7,
                 GSPLIT=2048):
        self.N, self.DIN, self.DH, self.DOUT = N, DIN, DH, DOUT
        self.NC, self.WPC, self.WPG, self.GSPLIT = NC, WPC, WPG, GSPLIT
        assert WPC % WPG == 0
        self.G = WPC // WPG
        self.NPC = WPC * 128            # slots per core
        self.TOT = NC * self.NPC
        self.PAIRS = self.TOT // 2
        assert self.PAIRS <= 32768      # int16 gather indices
        assert self.N <= self.TOT


FULL = Cfg()

# ---------------------------------------------------------------------------
# host-side graph prep (structure only)
# ---------------------------------------------------------------------------


def _assign_slots(load, cfg):
    """LPT-deal nodes into NC*WPC bins of <=128 slots, balancing `load`."""
    import heapq

    nbins = cfg.NC * cfg.WPC
    order = np.argsort(-load, kind="stable")
    heap = [(0, b) for b in range(nbins)]
    heapq.heapify(heap)
    count = np.zeros(nbins, np.int64)
    pos = np.empty(cfg.N, np.int64)
    for n in order:
        l, b = heapq.heappop(heap)
        pos[n] = b * 128 + count[b]
        count[b] += 1
        if count[b] < 128:
            heapq.heappush(heap, (l + int(load[n]), b))
    return pos


def prep(x, edge_index, cfg):
    N, NC, WPC, WPG, G = cfg.N, cfg.NC, cfg.WPC, cfg.WPG, cfg.G
    NPC, DIN = cfg.NPC, cfg.DIN

    src = edge_index[0].astype(np.int64)
    dst = edge_index[1].astype(np.int64)
    deg_in = np.bincount(dst, minlength=N).astype(np.int64)
    deg = (deg_in + 1).astype(np.float32)          # + self-loop
    dinv = (1.0 / np.sqrt(np.maximum(deg, 1e-12))).astype(np.float32)

    pos = _assign_slots(deg_in, cfg)

    node_of = np.full(cfg.TOT, -1, np.int64)
    node_of[pos] = np.arange(N)
    dinv_slot = np.ones(cfg.TOT, np.float32)
    dinv_slot[pos] = dinv

    # edge records (no self-loops)
    s_pos = pos[src]
    d_pos = pos[dst]
    c_e = d_pos // NPC
    w_e = (d_pos % NPC) // 128
    dval_e = (d_pos % 128 + 2).astype(np.float32)
    h_e = s_pos & 1
    gidx_e = s_pos >> 1

    key_all = (c_e * WPC + w_e) * 2 + h_e
    order_e = np.argsort(key_all, kind="stable")
    ks = key_all[order_e]
    bounds = np.searchsorted(ks, np.arange(NC * WPC * 2 + 1))
    buckets = {}
    for key in range(NC * WPC * 2):
        lo, hi = bounds[key], bounds[key + 1]
        if hi > lo:
            buckets[key] = order_e[lo:hi]

    tgt = np.zeros((WPC, 2), np.int64)
    for w in range(WPC):
        for h in range(2):
            mx = max(len(buckets.get((c * WPC + w) * 2 + h, ()))
                     for c in range(NC))
            tgt[w, h] = int(np.ceil(max(mx, 1) / 128) * 128)

    seglen = np.zeros((G, 2), np.int64)
    for g in range(G):
        for h in range(2):
            seglen[g, h] = tgt[g * WPG:(g + 1) * WPG, h].sum()

    idx_cols = int(sum(int(seglen[g, h]) // 16
                       for g in range(G) for h in range(2)))
    chunk_tot = int(sum(int(seglen[g, h]) // 128
                        for g in range(G) for h in range(2)))
    ioff, coff = {}, {}
    io = co = 0
    for g in range(G):
        for h in range(2):
            ioff[(g, h)] = io
            coff[(g, h)] = co
            io += int(seglen[g, h]) // 16
            co += int(seglen[g, h]) // 128
    wcol = np.zeros((WPC, 2), np.int64)
    for g in range(G):
        for h in range(2):
            c0 = coff[(g, h)]
            for wl in range(WPG):
                w = g * WPG + wl
                wcol[w, h] = c0
                c0 += int(tgt[w, h]) // 128

    import ml_dtypes
    bf16 = ml_dtypes.bfloat16
    ix_all = np.zeros((NC, 128, idx_cols), np.int16)
    dvl_all = np.full((NC, 128, chunk_tot), -1.0, np.float32)
    for c in range(NC):
        for g in range(G):
            for h in range(2):
                n = int(seglen[g, h])
                gi = np.zeros(n, np.int64)
                dv = np.full(n, -1.0, np.float32)
                p = 0
                for wl in range(WPG):
                    w = g * WPG + wl
                    es = buckets.get((c * WPC + w) * 2 + h, ())
                    ne = len(es)
                    if ne:
                        es = np.asarray(es)[np.argsort(gidx_e[es],
                                                       kind="stable")]
                    gi[p:p + ne] = gidx_e[es]
                    dv[p:p + ne] = dval_e[es]
                    p += int(tgt[w, h])
                wrapped = gi.reshape(n // 16, 16).T.astype(np.int16)
                ix_all[c, :, ioff[(g, h)]:ioff[(g, h)] + n // 16] = np.tile(
                    wrapped, (8, 1))
                dvl_all[c, :, coff[(g, h)]:coff[(g, h)] + n // 128] = (
                    dv.reshape(n // 128, 128).T)

    X_all = np.zeros((cfg.TOT, DIN), np.float32)
    X_all[pos] = np.asarray(x, np.float32)
    xT_sh = np.zeros((NC, DIN, NPC), bf16)
    dv128 = np.zeros((NC, 128, WPC), np.float32)
    dvrep = np.zeros((NC, 64, NPC), np.float32)
    for c in range(NC):
        sl = slice(c * NPC, (c + 1) * NPC)
        xT_sh[c] = X_all[sl].T.astype(bf16)
        ds = dinv_slot[sl]
        dv128[c] = ds.reshape(WPC, 128).T
        dvrep[c] = np.tile(ds, (64, 1))

    plan = dict(tgt=tgt, seglen=seglen, ioff=ioff, coff=coff, wcol=wcol,
                idx_cols=idx_cols, chunk_tot=chunk_tot)
    host = dict(xT_sh=xT_sh, dv128=dv128, dvrep=dvrep, ix_all=ix_all,
                dvl_all=dvl_all.astype(bf16), pos=pos)
    return plan, host


# ---------------------------------------------------------------------------
# device kernel
# ---------------------------------------------------------------------------


def build(cfg, plan):
    import concourse.bass as bass  # noqa: F401
    import concourse.mybir as mybir
    import concourse.tile as tile
    from concourse import bacc

    NC, WPC, WPG, G = cfg.NC, cfg.WPC, cfg.WPG, cfg.G
    NPC, TOT = cfg.NPC, cfg.TOT
    DH, DOUT = cfg.DH, cfg.DOUT
    f32 = mybir.dt.float32
    bf16 = mybir.dt.bfloat16
    AF = mybir.ActivationFunctionType
    tgt, seglen = plan["tgt"], plan["seglen"]
    ioff, coff, wcol = plan["ioff"], plan["coff"], plan["wcol"]

    nc = bacc.Bacc(
        "TRN2", target_bir_lowering=False, debug=False,
        num_devices=NC, num_swdge_queues=4,
    )

    xT_d = nc.dram_tensor("xT", [128, NPC], bf16, kind="ExternalInput")
    w1_d = nc.dram_tensor("w1", [128, DH], bf16, kind="ExternalInput")
    w2_d = nc.dram_tensor("w2", [DH, DOUT], bf16, kind="ExternalInput")
    dv128_d = nc.dram_tensor("dv128", [128, WPC], f32, kind="ExternalInput")
    dvrep_d = nc.dram_tensor("dvrep", [64, NPC], f32, kind="ExternalInput")
    id_d = nc.dram_tensor("ident", [128, 128], bf16, kind="ExternalInput")
    b1_d = nc.dram_tensor("b1c", [DH, 1], f32, kind="ExternalInput")
    b2_d = nc.dram_tensor("b2c", [DOUT, 1], f32, kind="ExternalInput")
    idaw_d = nc.dram_tensor("idaw", [DOUT, DOUT + 1], bf16,
                            kind="ExternalInput")
    ab_d = nc.dram_tensor("abc", [128, 1], f32, kind="ExternalInput")
    gi_d = nc.dram_tensor("gi128", [128, 128], bf16, kind="ExternalInput")
    ix_d = nc.dram_tensor("ix", [128, plan["idx_cols"]], mybir.dt.int16,
                          kind="ExternalInput")
    dvl_d = nc.dram_tensor("dvl", [128, plan["chunk_tot"]], bf16,
                           kind="ExternalInput")
    out_d = nc.dram_tensor("out_sh", [NPC, DOUT], f32, kind="ExternalOutput")

    rg = [list(range(NC))]
    qctr = [0]

    with tile.TileContext(nc) as tc:
        with tc.tile_pool(name="const", bufs=1) as cpool:
            def load(dram, shape, dt=f32):
                t = cpool.tile(shape, dt, tag=dram.name, name=dram.name + "_s")
                nc.sync.dma_start(t[:], dram.ap())
                return t

            w1_s = load(w1_d, [128, DH], bf16)
            w2_s = load(w2_d, [DH, DOUT], bf16)
            dv128_s = load(dv128_d, [128, WPC])
            dvrep_s = load(dvrep_d, [64, NPC])
            id_s = load(id_d, [128, 128], bf16)
            b1_s = load(b1_d, [DH, 1])
            b2_s = load(b2_d, [DOUT, 1])
            idaw_s = load(idaw_d, [DOUT, DOUT + 1], bf16)
            ab_s = load(ab_d, [128, 1])
            gi_s = load(gi_d, [128, 128], bf16)
            ix_s = load(ix_d, [128, plan["idx_cols"]], mybir.dt.int16)
            dvl_s = load(dvl_d, [128, plan["chunk_tot"]], bf16)

            t1sb = cpool.tile([128, WPC * 64], bf16, tag="t1sb", name="t1sb")
            t2sb = cpool.tile([128, WPC * 64], bf16, tag="t2sb", name="t2sb")
            h2at = cpool.tile([128, WPC * 33], f32, tag="h2at", name="h2at")
            nc.vector.memset(t2sb[:], 0.0)

            with tc.tile_pool(name="dram", bufs=1, space="DRAM") as dpool:
                t1_shard = dpool.tile([NPC, DH], bf16, tag="t1s", name="t1s")
                t1_full = dpool.tile([TOT, DH], bf16, tag="t1f", name="t1f",
                                     addr_space="Shared")
                t2_shard = dpool.tile([NPC, DH], bf16, tag="t2s", name="t2s")
                t2_full = dpool.tile([TOT, DH], bf16, tag="t2f", name="t2f",
                                     addr_space="Shared")

                # ---- phase 1: t1 = dinv .* (x @ W1), node-major bf16
                with (
                    tc.tile_pool(name="ph1", bufs=1) as ph1,
                    tc.tile_pool(name="ph1ps", bufs=4, space="PSUM") as pps,
                ):
                    xts = ph1.tile([128, NPC], bf16, tag="xts", name="xts")
                    nc.sync.dma_start(xts[:], xT_d.ap())
                    for b in range(WPC):
                        ps = pps.tile([128, DH], f32, tag="p1", name="p1")
                        nc.tensor.matmul(ps[:],
                                         lhsT=xts[:, b * 128:(b + 1) * 128],
                                         rhs=w1_s[:], start=True, stop=True)
                        nc.scalar.activation(
                            t1sb[:, b * 64:(b + 1) * 64], ps[:],
                            func=AF.Copy, scale=dv128_s[:, b:b + 1])
                        nc.sync.dma_start(
                            t1_shard[b * 128:(b + 1) * 128, :],
                            t1sb[:, b * 64:(b + 1) * 64])

                nc.gpsimd.collective_compute(
                    "AllGather", mybir.AluOpType.bypass, replica_groups=rg,
                    ins=[t1_shard[:]], outs=[t1_full[:]],
                )

                # ---- shared aggregation loop
                def aggregate(full, used, sl_lhs, flush_fn):
                    fv = full.rearrange("(a b) d -> a (b d)", b=2)
                    with (
                        tc.tile_pool(name="gp", bufs=2) as gp,
                        tc.tile_pool(name="sp", bufs=2) as sp,
                        tc.tile_pool(name="aps", bufs=4, space="PSUM") as aps,
                        tc.tile_pool(name="fsb", bufs=3) as fsb,
                        tc.tile_pool(name="fps", bufs=2, space="PSUM") as fps,
                    ):
                        for g in range(G):
                            gts, Ss = {}, {}
                            for h in range(2):
                                n = int(seglen[g, h])
                                nch = n // 128
                                gt = gp.tile([128, n], bf16, tag=f"g{h}",
                                             name=f"gt{h}")
                                io = ioff[(g, h)]
                                o0 = 0
                                while o0 < n:
                                    nn = min(cfg.GSPLIT, n - o0)
                                    nc.gpsimd.dma_gather(
                                        out_ap=gt[:, o0:o0 + nn].rearrange(
                                            "p (c d) -> p c d", d=128),
                                        in_ap=fv,
                                        idxs_ap=ix_s[:, io + o0 // 16:
                                                     io + (o0 + nn) // 16],
                                        num_idxs=nn, num_idxs_reg=nn,
                                        elem_size=128, elem_step=128,
                                        queue_num=qctr[0] % 4,
                                        single_packet=False,
                                    )
                                    qctr[0] += 1
                                    o0 += nn
                                c0 = coff[(g, h)]
                                S = sp.tile([128, nch * 128], bf16,
                                            tag=f"S{h}", name=f"S{h}")
                                nc.vector.tensor_tensor(
                                    out=S[:].rearrange(
                                        "p (c j) -> p c j", j=128),
                                    in0=dvl_s[:, c0:c0 + nch].unsqueeze(2)
                                    .broadcast_to((128, nch, 128)),
                                    in1=gi_s[:].unsqueeze(1)
                                    .broadcast_to((128, nch, 128)),
                                    op=mybir.AluOpType.is_equal,
                                )
                                gts[h], Ss[h] = gt, S
                            for wl in range(WPG):
                                w = g * WPG + wl
                                ps = aps.tile([used, 128], f32, tag="agg",
                                              name="agg")
                                nc.tensor.matmul(
                                    ps[:], lhsT=sl_lhs(w), rhs=id_s[:],
                                    start=True, stop=False)
                                chunks = (
                                    [(0, k)
                                     for k in range(int(tgt[w, 0]) // 128)]
                                    + [(1, k)
                                       for k in range(int(tgt[w, 1]) // 128)])
                                for j, (h, k) in enumerate(chunks):
                                    kk = int(wcol[w, h] - coff[(g, h)]) + k
                                    base = kk * 128 + h * 64
                                    nc.tensor.matmul(
                                        ps[:],
                                        lhsT=gts[h][:, base:base + used],
                                        rhs=Ss[h][:, kk * 128:(kk + 1) * 128],
                                        start=False,
                                        stop=(j == len(chunks) - 1))
                                flush_fn(w, ps, fsb, fps)

                # ---- layer-1 flush
                def flush1(w, ps, fsb, fps):
                    a = fsb.tile([64, 128], f32, tag="a", name="a")
                    nc.vector.tensor_tensor(
                        out=a[:], in0=ps[:],
                        in1=dvrep_s[:, w * 128:(w + 1) * 128],
                        op=mybir.AluOpType.mult)
                    hT = fsb.tile([64, 128], bf16, tag="hT", name="hT")
                    nc.scalar.activation(hT[:], a[:], func=AF.Relu,
                                         bias=b1_s[:, 0:1])
                    t2ps = fps.tile([128, DOUT], f32, tag="t2ps", name="t2ps")
                    nc.tensor.matmul(t2ps[:], lhsT=hT[:], rhs=w2_s[:],
                                     start=True, stop=True)
                    nc.scalar.activation(
                        t2sb[:, w * 64:w * 64 + DOUT], t2ps[:],
                        func=AF.Copy, scale=dv128_s[:, w:w + 1])
                    nc.sync.dma_start(
                        t2_shard[w * 128:(w + 1) * 128, :],
                        t2sb[:, w * 64:(w + 1) * 64])

                def sl1(w):
                    return t1sb[:, w * 64:(w + 1) * 64]

                aggregate(t1_full[:], DH, sl1, flush1)

                nc.gpsimd.collective_compute(
                    "AllGather", mybir.AluOpType.bypass, replica_groups=rg,
                    ins=[t2_shard[:]], outs=[t2_full[:]],
                )

                # ---- layer-2 flush
                def flush2(w, ps, fsb, fps):
                    a2 = fsb.tile([DOUT, 128], f32, tag="a2", name="a2")
                    nc.vector.tensor_tensor(
                        out=a2[:], in0=ps[:],
                        in1=dvrep_s[:DOUT, w * 128:(w + 1) * 128],
                        op=mybir.AluOpType.mult)
                    h2T = fsb.tile([DOUT, 128], bf16, tag="h2T", name="h2T")
                    nc.scalar.activation(h2T[:], a2[:], func=AF.Relu,
                                         bias=b2_s[:, 0:1])
                    gps = fps.tile([128, DOUT + 1], f32, tag="gps", name="gps")
                    nc.tensor.matmul(gps[:], lhsT=h2T[:], rhs=idaw_s[:],
                                     start=True, stop=True)
                    nc.scalar.activation(
                        h2at[:, w * 33:(w + 1) * 33], gps[:], func=AF.Copy)

                def sl2(w):
                    return t2sb[:, w * 64:w * 64 + DOUT]

                aggregate(t2_full[:], DOUT, sl2, flush2)

                # ---- attention gate tail
                with tc.tile_pool(name="tail", bufs=1) as tp:
                    atall = tp.tile([128, WPC], f32, tag="atall", name="atall")
                    nc.scalar.activation(
                        atall[:],
                        h2at[:].rearrange("p (w q) -> p w q", q=33)[:, :, 32],
                        func=AF.Sigmoid, bias=ab_s[:, 0:1])
                    oall = tp.tile([128, WPC * DOUT], f32, tag="oall",
                                   name="oall")
                    nc.vector.tensor_tensor(
                        out=oall[:].rearrange("p (w f) -> p w f", f=DOUT),
                        in0=h2at[:].rearrange(
                            "p (w q) -> p w q", q=33)[:, :, 0:DOUT],
                        in1=atall[:].unsqueeze(2)
                        .broadcast_to((128, WPC, DOUT)),
                        op=mybir.AluOpType.mult)
                    nc.sync.dma_start(
                        out_d.ap().rearrange("(w p) f -> p w f", p=128),
                        oall[:].rearrange("p (w f) -> p w f", f=DOUT))

    nc.compile()
    return nc


# ---------------------------------------------------------------------------
# entry point
# ---------------------------------------------------------------------------


def _make_in_maps(cfg, host, W1, b1, W2, b2, attn_w, attn_b):
    import ml_dtypes
    bf16 = ml_dtypes.bfloat16
    giota = np.tile(np.arange(2, 130, dtype=np.float32),
                    (128, 1)).astype(bf16)
    idaw = np.concatenate(
        [np.eye(cfg.DOUT, dtype=np.float32),
         np.asarray(attn_w, np.float32).reshape(cfg.DOUT, 1)],
        axis=1).astype(bf16)
    in_maps = []
    for c in range(cfg.NC):
        in_maps.append({
            "xT": host["xT_sh"][c],
            "w1": np.asarray(W1, np.float32).astype(bf16),
            "w2": np.asarray(W2, np.float32).astype(bf16),
            "dv128": host["dv128"][c],
            "dvrep": host["dvrep"][c],
            "ident": np.eye(128, dtype=np.float32).astype(bf16),
            "b1c": np.asarray(b1, np.float32).reshape(cfg.DH, 1),
            "b2c": np.asarray(b2, np.float32).reshape(cfg.DOUT, 1),
            "idaw": idaw,
            "abc": np.full((128, 1),
                           np.asarray(attn_b, np.float32).reshape(-1)[0],
                           np.float32),
            "gi128": giota,
            "ix": host["ix_all"][c],
            "dvl": host["dvl_all"][c],
        })
    return in_maps


def run(x, edge_index, W1, b1, W2, b2, attn_w, attn_b, cfg=None,
        backend="hw", trace=False):
    cfg = cfg or FULL
    plan, host = prep(x, edge_index, cfg)
    nc = build(cfg, plan)
    in_maps = _make_in_maps(cfg, host, W1, b1, W2, b2, attn_w, attn_b)

    if backend == "sim":
        from concourse.bass_interp import MultiCoreSim
        sim = MultiCoreSim(nc, num_cores=cfg.NC, trace=False)
        for c, core in enumerate(sim.cores.values()):
            for name, arr in in_maps[c].items():
                core.tensor(name)[:] = arr
        sim.simulate()
        outs = [core.tensor("out_sh").copy() for core in sim.cores.values()]
        exec_ns = None
    else:
        from concourse import bass_utils
        from concourse.bass_interp import get_hw_module
        old = nc.m
        nc.m = get_hw_module(nc.m)
        try:
            res = bass_utils.run_bass_kernel_spmd(
                nc, in_maps, core_ids=list(range(cfg.NC)), trace=trace)
        finally:
            nc.m = old
        outs = [res.results[c]["out_sh"] for c in range(cfg.NC)]
        exec_ns = res.exec_time_ns

    full = np.concatenate(outs, axis=0)   # [TOT, DOUT] in slot order
    out = full[host["pos"]]               # unpermute -> [N, DOUT]
    return np.ascontiguousarray(out), exec_ns


def kernel(x, edge_index, W1, b1, W2, b2, attn_w, attn_b):
    out, _ = run(x, edge_index, W1, b1, W2, b2, attn_w, attn_b,
                 cfg=FULL, backend="hw", trace=False)
    return out


# revision 6
# speedup vs baseline: 1.6239x; 1.0059x over previous
"""Trainium2 Bass kernel v3 for a 2-layer GCN with data-aware attention gate.

Math (per reference):
    src,dst = edges + self-loops; deg = bincount(dst); dinv = rsqrt(deg)
    norm = dinv[src]*dinv[dst]
    h1 = relu(segsum(norm * (x@W1)[src], dst) + b1)
    h2 = relu(segsum(norm * (h1@W2)[src], dst) + b2)
    out = h2 * sigmoid(h2@attn_w + attn_b)

v3 design (8 NeuronCores, dst-sharded, 128-slot windows):
  - Tables are pure bf16, node-major, pair-packed: gather element = two
    consecutive slots' rows = 256B.  Both layers share ONE edge plan:
    idx = s_pos>>1 (int16-safe), parity h = s_pos&1 selects the element
    half.  Layer-1 rows are 64 bf16 feats; layer-2 rows [32 feats|32 z].
  - Gathers are split into <=2048-element pieces across the 4 SWDGE
    queues so the 128-entry descriptor FIFO never starves the SDMA
    engines (measured 2.5x drain-rate difference).
  - Transposed aggregation: matmul(lhsT=gathered-chunk[128e x used],
    rhs=S[128e x 128 slots]) accumulates feat-major [used, 128] PSUM per
    window; no per-window PE transposes.
  - S one-hot matrices built ONE wide is_equal per gather-group via
    stride-0 broadcast APs (dval col vs iota row).
  - Self-loops excluded from edge lists; handled as one PE matmul per
    window: lhsT = own shard rows (SBUF-resident), rhs = identity.
  - All per-partition scalings ride scalar-engine ACT scale/bias.
"""

import sys

import numpy as np

_CONC = "/opt/trn_rl_repo"
if _CONC not in sys.path:
    sys.path.insert(0, _CONC)

# ---------------------------------------------------------------------------
# configuration
# ---------------------------------------------------------------------------


class Cfg:
    def __init__(self, N=50000, DIN=128, DH=64, DOUT=32, NC=8, WPC=49, WPG=5,
                 GSPLIT=2048):
        self.N, self.DIN, self.DH, self.DOUT = N, DIN, DH, DOUT
        self.NC, self.WPC, self.WPG, self.GSPLIT = NC, WPC, WPG, GSPLIT
        # variable-size window groups: WPG windows each, remainder in last
        self.groups = []
        w0 = 0
        while w0 < WPC:
            nw = min(WPG, WPC - w0)
            self.groups.append((w0, nw))
            w0 += nw
        self.G = len(self.groups)
        self.NPC = WPC * 128            # slots per core
        self.TOT = NC * self.NPC
        self.PAIRS = self.TOT // 2
        assert self.PAIRS <= 32768      # int16 gather indices
        assert self.N <= self.TOT


FULL = Cfg()

# ---------------------------------------------------------------------------
# host-side graph prep (structure only)
# ---------------------------------------------------------------------------


def _assign_slots(load, cfg):
    """LPT-deal nodes into NC*WPC bins of <=128 slots, balancing `load`."""
    import heapq

    nbins = cfg.NC * cfg.WPC
    order = np.argsort(-load, kind="stable")
    heap = [(0, b) for b in range(nbins)]
    heapq.heapify(heap)
    count = np.zeros(nbins, np.int64)
    pos = np.empty(cfg.N, np.int64)
    for n in order:
        l, b = heapq.heappop(heap)
        pos[n] = b * 128 + count[b]
        count[b] += 1
        if count[b] < 128:
            heapq.heappush(heap, (l + int(load[n]), b))
    return pos


def prep(x, edge_index, cfg):
    N, NC, WPC, WPG, G = cfg.N, cfg.NC, cfg.WPC, cfg.WPG, cfg.G
    NPC, DIN = cfg.NPC, cfg.DIN

    src = edge_index[0].astype(np.int64)
    dst = edge_index[1].astype(np.int64)
    deg_in = np.bincount(dst, minlength=N).astype(np.int64)
    deg = (deg_in + 1).astype(np.float32)          # + self-loop
    dinv = (1.0 / np.sqrt(np.maximum(deg, 1e-12))).astype(np.float32)

    pos = _assign_slots(deg_in, cfg)

    node_of = np.full(cfg.TOT, -1, np.int64)
    node_of[pos] = np.arange(N)
    dinv_slot = np.ones(cfg.TOT, np.float32)
    dinv_slot[pos] = dinv

    # edge records (no self-loops)
    s_pos = pos[src]
    d_pos = pos[dst]
    c_e = d_pos // NPC
    w_e = (d_pos % NPC) // 128
    dval_e = (d_pos % 128 + 2).astype(np.float32)
    h_e = s_pos & 1
    gidx_e = s_pos >> 1

    key_all = (c_e * WPC + w_e) * 2 + h_e
    order_e = np.argsort(key_all, kind="stable")
    ks = key_all[order_e]
    bounds = np.searchsorted(ks, np.arange(NC * WPC * 2 + 1))
    buckets = {}
    for key in range(NC * WPC * 2):
        lo, hi = bounds[key], bounds[key + 1]
        if hi > lo:
            buckets[key] = order_e[lo:hi]

    tgt = np.zeros((WPC, 2), np.int64)
    for w in range(WPC):
        for h in range(2):
            mx = max(len(buckets.get((c * WPC + w) * 2 + h, ()))
                     for c in range(NC))
            tgt[w, h] = int(np.ceil(max(mx, 1) / 128) * 128)

    seglen = np.zeros((G, 2), np.int64)
    for g, (w0, nw) in enumerate(cfg.groups):
        for h in range(2):
            seglen[g, h] = tgt[w0:w0 + nw, h].sum()

    idx_cols = int(sum(int(seglen[g, h]) // 16
                       for g in range(G) for h in range(2)))
    chunk_tot = int(sum(int(seglen[g, h]) // 128
                        for g in range(G) for h in range(2)))
    ioff, coff = {}, {}
    io = co = 0
    for g in range(G):
        for h in range(2):
            ioff[(g, h)] = io
            coff[(g, h)] = co
            io += int(seglen[g, h]) // 16
            co += int(seglen[g, h]) // 128
    wcol = np.zeros((WPC, 2), np.int64)
    for g, (w0, nw) in enumerate(cfg.groups):
        for h in range(2):
            c0 = coff[(g, h)]
            for wl in range(nw):
                w = w0 + wl
                wcol[w, h] = c0
                c0 += int(tgt[w, h]) // 128

    import ml_dtypes
    bf16 = ml_dtypes.bfloat16
    ix_all = np.zeros((NC, 128, idx_cols), np.int16)
    dvl_all = np.full((NC, 128, chunk_tot), -1.0, np.float32)
    for c in range(NC):
        for g, (w0, nw) in enumerate(cfg.groups):
            for h in range(2):
                n = int(seglen[g, h])
                gi = np.zeros(n, np.int64)
                dv = np.full(n, -1.0, np.float32)
                p = 0
                for wl in range(nw):
                    w = w0 + wl
                    es = buckets.get((c * WPC + w) * 2 + h, ())
                    ne = len(es)
                    if ne:
                        es = np.asarray(es)[np.argsort(gidx_e[es],
                                                       kind="stable")]
                    gi[p:p + ne] = gidx_e[es]
                    dv[p:p + ne] = dval_e[es]
                    p += int(tgt[w, h])
                wrapped = gi.reshape(n // 16, 16).T.astype(np.int16)
                ix_all[c, :, ioff[(g, h)]:ioff[(g, h)] + n // 16] = np.tile(
                    wrapped, (8, 1))
                dvl_all[c, :, coff[(g, h)]:coff[(g, h)] + n // 128] = (
                    dv.reshape(n // 128, 128).T)

    X_all = np.zeros((cfg.TOT, DIN), np.float32)
    X_all[pos] = np.asarray(x, np.float32)
    xT_sh = np.zeros((NC, DIN, NPC), bf16)
    dv128 = np.zeros((NC, 128, WPC), np.float32)
    dvrep = np.zeros((NC, 64, NPC), np.float32)
    for c in range(NC):
        sl = slice(c * NPC, (c + 1) * NPC)
        xT_sh[c] = X_all[sl].T.astype(bf16)
        ds = dinv_slot[sl]
        dv128[c] = ds.reshape(WPC, 128).T
        dvrep[c] = np.tile(ds, (64, 1))

    plan = dict(tgt=tgt, seglen=seglen, ioff=ioff, coff=coff, wcol=wcol,
                idx_cols=idx_cols, chunk_tot=chunk_tot)
    host = dict(xT_sh=xT_sh, dv128=dv128, dvrep=dvrep, ix_all=ix_all,
                dvl_all=dvl_all.astype(bf16), pos=pos)
    return plan, host


# ---------------------------------------------------------------------------
# device kernel
# ---------------------------------------------------------------------------


def build(cfg, plan):
    import concourse.bass as bass  # noqa: F401
    import concourse.mybir as mybir
    import concourse.tile as tile
    from concourse import bacc

    NC, WPC, WPG, G = cfg.NC, cfg.WPC, cfg.WPG, cfg.G
    NPC, TOT = cfg.NPC, cfg.TOT
    DH, DOUT = cfg.DH, cfg.DOUT
    f32 = mybir.dt.float32
    bf16 = mybir.dt.bfloat16
    AF = mybir.ActivationFunctionType
    tgt, seglen = plan["tgt"], plan["seglen"]
    ioff, coff, wcol = plan["ioff"], plan["coff"], plan["wcol"]

    nc = bacc.Bacc(
        "TRN2", target_bir_lowering=False, debug=False,
        num_devices=NC, num_swdge_queues=4,
    )

    xT_d = nc.dram_tensor("xT", [128, NPC], bf16, kind="ExternalInput")
    w1_d = nc.dram_tensor("w1", [128, DH], bf16, kind="ExternalInput")
    w2_d = nc.dram_tensor("w2", [DH, DOUT], bf16, kind="ExternalInput")
    dv128_d = nc.dram_tensor("dv128", [128, WPC], f32, kind="ExternalInput")
    dvrep_d = nc.dram_tensor("dvrep", [64, NPC], f32, kind="ExternalInput")
    id_d = nc.dram_tensor("ident", [128, 128], bf16, kind="ExternalInput")
    b1_d = nc.dram_tensor("b1c", [DH, 1], f32, kind="ExternalInput")
    b2_d = nc.dram_tensor("b2c", [DOUT, 1], f32, kind="ExternalInput")
    idaw_d = nc.dram_tensor("idaw", [DOUT, DOUT + 1], bf16,
                            kind="ExternalInput")
    ab_d = nc.dram_tensor("abc", [128, 1], f32, kind="ExternalInput")
    gi_d = nc.dram_tensor("gi128", [128, 128], bf16, kind="ExternalInput")
    ix_d = nc.dram_tensor("ix", [128, plan["idx_cols"]], mybir.dt.int16,
                          kind="ExternalInput")
    dvl_d = nc.dram_tensor("dvl", [128, plan["chunk_tot"]], bf16,
                           kind="ExternalInput")
    out_d = nc.dram_tensor("out_sh", [NPC, DOUT], f32, kind="ExternalOutput")

    rg = [list(range(NC))]
    qctr = [0]

    with tile.TileContext(nc) as tc:
        with tc.tile_pool(name="const", bufs=1) as cpool:
            def load(dram, shape, dt=f32):
                t = cpool.tile(shape, dt, tag=dram.name, name=dram.name + "_s")
                nc.sync.dma_start(t[:], dram.ap())
                return t

            w1_s = load(w1_d, [128, DH], bf16)
            w2_s = load(w2_d, [DH, DOUT], bf16)
            dv128_s = load(dv128_d, [128, WPC])
            dvrep_s = load(dvrep_d, [64, NPC])
            id_s = load(id_d, [128, 128], bf16)
            b1_s = load(b1_d, [DH, 1])
            b2_s = load(b2_d, [DOUT, 1])
            idaw_s = load(idaw_d, [DOUT, DOUT + 1], bf16)
            ab_s = load(ab_d, [128, 1])
            gi_s = load(gi_d, [128, 128], bf16)
            ix_s = load(ix_d, [128, plan["idx_cols"]], mybir.dt.int16)
            dvl_s = load(dvl_d, [128, plan["chunk_tot"]], bf16)

            t1sb = cpool.tile([128, WPC * 64], bf16, tag="t1sb", name="t1sb")
            t2sb = cpool.tile([128, WPC * 64], bf16, tag="t2sb", name="t2sb")
            h2at = cpool.tile([128, WPC * 33], f32, tag="h2at", name="h2at")
            nc.vector.memset(t2sb[:], 0.0)

            with tc.tile_pool(name="dram", bufs=1, space="DRAM") as dpool:
                t1_shard = dpool.tile([NPC, DH], bf16, tag="t1s", name="t1s")
                t1_full = dpool.tile([TOT, DH], bf16, tag="t1f", name="t1f",
                                     addr_space="Shared")
                t2_shard = dpool.tile([NPC, DH], bf16, tag="t2s", name="t2s")
                t2_full = dpool.tile([TOT, DH], bf16, tag="t2f", name="t2f",
                                     addr_space="Shared")

                # ---- phase 1: t1 = dinv .* (x @ W1), node-major bf16
                with (
                    tc.tile_pool(name="ph1", bufs=1) as ph1,
                    tc.tile_pool(name="ph1ps", bufs=4, space="PSUM") as pps,
                ):
                    xts = ph1.tile([128, NPC], bf16, tag="xts", name="xts")
                    nc.sync.dma_start(xts[:], xT_d.ap())
                    for b in range(WPC):
                        ps = pps.tile([128, DH], f32, tag="p1", name="p1")
                        nc.tensor.matmul(ps[:],
                                         lhsT=xts[:, b * 128:(b + 1) * 128],
                                         rhs=w1_s[:], start=True, stop=True)
                        nc.scalar.activation(
                            t1sb[:, b * 64:(b + 1) * 64], ps[:],
                            func=AF.Copy, scale=dv128_s[:, b:b + 1])
                    nc.sync.dma_start(
                        t1_shard.rearrange("(w p) f -> p w f", p=128),
                        t1sb[:].rearrange("p (w f) -> p w f", f=64))

                nc.gpsimd.collective_compute(
                    "AllGather", mybir.AluOpType.bypass, replica_groups=rg,
                    ins=[t1_shard[:]], outs=[t1_full[:]],
                )

                # ---- shared aggregation loop
                def aggregate(full, used, sl_lhs, flush_fn):
                    fv = full.rearrange("(a b) d -> a (b d)", b=2)
                    with (
                        tc.tile_pool(name="gp", bufs=3) as gp,
                        tc.tile_pool(name="sp", bufs=2) as sp,
                        tc.tile_pool(name="aps", bufs=4, space="PSUM") as aps,
                        tc.tile_pool(name="fsb", bufs=3) as fsb,
                        tc.tile_pool(name="fps", bufs=2, space="PSUM") as fps,
                    ):
                        for g, (w0, nw) in enumerate(cfg.groups):
                            gts, Ss = {}, {}
                            for h in range(2):
                                n = int(seglen[g, h])
                                nch = n // 128
                                gt = gp.tile([128, n], bf16, tag=f"g{h}",
                                             name=f"gt{h}")
                                io = ioff[(g, h)]
                                o0 = 0
                                while o0 < n:
                                    nn = min(cfg.GSPLIT, n - o0)
                                    nc.gpsimd.dma_gather(
                                        out_ap=gt[:, o0:o0 + nn].rearrange(
                                            "p (c d) -> p c d", d=128),
                                        in_ap=fv,
                                        idxs_ap=ix_s[:, io + o0 // 16:
                                                     io + (o0 + nn) // 16],
                                        num_idxs=nn, num_idxs_reg=nn,
                                        elem_size=128, elem_step=128,
                                        queue_num=qctr[0] % 4,
                                        single_packet=False,
                                    )
                                    qctr[0] += 1
                                    o0 += nn
                                c0 = coff[(g, h)]
                                S = sp.tile([128, nch * 128], bf16,
                                            tag=f"S{h}", name=f"S{h}")
                                nc.vector.tensor_tensor(
                                    out=S[:].rearrange(
                                        "p (c j) -> p c j", j=128),
                                    in0=dvl_s[:, c0:c0 + nch].unsqueeze(2)
                                    .broadcast_to((128, nch, 128)),
                                    in1=gi_s[:].unsqueeze(1)
                                    .broadcast_to((128, nch, 128)),
                                    op=mybir.AluOpType.is_equal,
                                )
                                gts[h], Ss[h] = gt, S
                            for wl in range(nw):
                                w = w0 + wl
                                ps = aps.tile([used, 128], f32, tag="agg",
                                              name="agg")
                                nc.tensor.matmul(
                                    ps[:], lhsT=sl_lhs(w), rhs=id_s[:],
                                    start=True, stop=False)
                                chunks = (
                                    [(0, k)
                                     for k in range(int(tgt[w, 0]) // 128)]
                                    + [(1, k)
                                       for k in range(int(tgt[w, 1]) // 128)])
                                for j, (h, k) in enumerate(chunks):
                                    kk = int(wcol[w, h] - coff[(g, h)]) + k
                                    base = kk * 128 + h * 64
                                    nc.tensor.matmul(
                                        ps[:],
                                        lhsT=gts[h][:, base:base + used],
                                        rhs=Ss[h][:, kk * 128:(kk + 1) * 128],
                                        start=False,
                                        stop=(j == len(chunks) - 1))
                                flush_fn(w, ps, fsb, fps)

                # ---- layer-1 flush
                def flush1(w, ps, fsb, fps):
                    a = fsb.tile([64, 128], f32, tag="a", name="a")
                    nc.vector.tensor_tensor(
                        out=a[:], in0=ps[:],
                        in1=dvrep_s[:, w * 128:(w + 1) * 128],
                        op=mybir.AluOpType.mult)
                    hT = fsb.tile([64, 128], bf16, tag="hT", name="hT")
                    nc.scalar.activation(hT[:], a[:], func=AF.Relu,
                                         bias=b1_s[:, 0:1])
                    t2ps = fps.tile([128, DOUT], f32, tag="t2ps", name="t2ps")
                    nc.tensor.matmul(t2ps[:], lhsT=hT[:], rhs=w2_s[:],
                                     start=True, stop=True)
                    nc.scalar.activation(
                        t2sb[:, w * 64:w * 64 + DOUT], t2ps[:],
                        func=AF.Copy, scale=dv128_s[:, w:w + 1])

                def sl1(w):
                    return t1sb[:, w * 64:(w + 1) * 64]

                aggregate(t1_full[:], DH, sl1, flush1)
                nc.sync.dma_start(
                    t2_shard.rearrange("(w p) f -> p w f", p=128),
                    t2sb[:].rearrange("p (w f) -> p w f", f=64))

                nc.gpsimd.collective_compute(
                    "AllGather", mybir.AluOpType.bypass, replica_groups=rg,
                    ins=[t2_shard[:]], outs=[t2_full[:]],
                )

                # ---- layer-2 flush
                def flush2(w, ps, fsb, fps):
                    a2 = fsb.tile([DOUT, 128], f32, tag="a2", name="a2")
                    nc.vector.tensor_tensor(
                        out=a2[:], in0=ps[:],
                        in1=dvrep_s[:DOUT, w * 128:(w + 1) * 128],
                        op=mybir.AluOpType.mult)
                    h2T = fsb.tile([DOUT, 128], bf16, tag="h2T", name="h2T")
                    nc.scalar.activation(h2T[:], a2[:], func=AF.Relu,
                                         bias=b2_s[:, 0:1])
                    gps = fps.tile([128, DOUT + 1], f32, tag="gps", name="gps")
                    nc.tensor.matmul(gps[:], lhsT=h2T[:], rhs=idaw_s[:],
                                     start=True, stop=True)
                    nc.scalar.activation(
                        h2at[:, w * 33:(w + 1) * 33], gps[:], func=AF.Copy)

                def sl2(w):
                    return t2sb[:, w * 64:w * 64 + DOUT]

                aggregate(t2_full[:], DOUT, sl2, flush2)

                # ---- attention gate tail
                with tc.tile_pool(name="tail", bufs=1) as tp:
                    atall = tp.tile([128, WPC], f32, tag="atall", name="atall")
                    nc.scalar.activation(
                        atall[:],
                        h2at[:].rearrange("p (w q) -> p w q", q=33)[:, :, 32],
                        func=AF.Sigmoid, bias=ab_s[:, 0:1])
                    oall = tp.tile([128, WPC * DOUT], f32, tag="oall",
                                   name="oall")
                    nc.vector.tensor_tensor(
                        out=oall[:].rearrange("p (w f) -> p w f", f=DOUT),
                        in0=h2at[:].rearrange(
                            "p (w q) -> p w q", q=33)[:, :, 0:DOUT],
                        in1=atall[:].unsqueeze(2)
                        .broadcast_to((128, WPC, DOUT)),
                        op=mybir.AluOpType.mult)
                    nc.sync.dma_start(
                        out_d.ap().rearrange("(w p) f -> p w f", p=128),
                        oall[:].rearrange("p (w f) -> p w f", f=DOUT))

    nc.compile()
    return nc


# ---------------------------------------------------------------------------
# entry point
# ---------------------------------------------------------------------------


def _make_in_maps(cfg, host, W1, b1, W2, b2, attn_w, attn_b):
    import ml_dtypes
    bf16 = ml_dtypes.bfloat16
    giota = np.tile(np.arange(2, 130, dtype=np.float32),
                    (128, 1)).astype(bf16)
    idaw = np.concatenate(
        [np.eye(cfg.DOUT, dtype=np.float32),
         np.asarray(attn_w, np.float32).reshape(cfg.DOUT, 1)],
        axis=1).astype(bf16)
    in_maps = []
    for c in range(cfg.NC):
        in_maps.append({
            "xT": host["xT_sh"][c],
            "w1": np.asarray(W1, np.float32).astype(bf16),
            "w2": np.asarray(W2, np.float32).astype(bf16),
            "dv128": host["dv128"][c],
            "dvrep": host["dvrep"][c],
            "ident": np.eye(128, dtype=np.float32).astype(bf16),
            "b1c": np.asarray(b1, np.float32).reshape(cfg.DH, 1),
            "b2c": np.asarray(b2, np.float32).reshape(cfg.DOUT, 1),
            "idaw": idaw,
            "abc": np.full((128, 1),
                           np.asarray(attn_b, np.float32).reshape(-1)[0],
                           np.float32),
            "gi128": giota,
            "ix": host["ix_all"][c],
            "dvl": host["dvl_all"][c],
        })
    return in_maps


def run(x, edge_index, W1, b1, W2, b2, attn_w, attn_b, cfg=None,
        backend="hw", trace=False):
    cfg = cfg or FULL
    plan, host = prep(x, edge_index, cfg)
    nc = build(cfg, plan)
    in_maps = _make_in_maps(cfg, host, W1, b1, W2, b2, attn_w, attn_b)

    if backend == "sim":
        from concourse.bass_interp import MultiCoreSim
        sim = MultiCoreSim(nc, num_cores=cfg.NC, trace=False)
        for c, core in enumerate(sim.cores.values()):
            for name, arr in in_maps[c].items():
                core.tensor(name)[:] = arr
        sim.simulate()
        outs = [core.tensor("out_sh").copy() for core in sim.cores.values()]
        exec_ns = None
    else:
        from concourse import bass_utils
        from concourse.bass_interp import get_hw_module
        old = nc.m
        nc.m = get_hw_module(nc.m)
        try:
            res = bass_utils.run_bass_kernel_spmd(
                nc, in_maps, core_ids=list(range(cfg.NC)), trace=trace)
        finally:
            nc.m = old
        outs = [res.results[c]["out_sh"] for c in range(cfg.NC)]
        exec_ns = res.exec_time_ns

    full = np.concatenate(outs, axis=0)   # [TOT, DOUT] in slot order
    out = full[host["pos"]]               # unpermute -> [N, DOUT]
    return np.ascontiguousarray(out), exec_ns


def kernel(x, edge_index, W1, b1, W2, b2, attn_w, attn_b):
    out, _ = run(x, edge_index, W1, b1, W2, b2, attn_w, attn_b,
                 cfg=FULL, backend="hw", trace=False)
    return out


# revision 7
# speedup vs baseline: 1.6559x; 1.0197x over previous
"""Trainium2 Bass kernel v3 for a 2-layer GCN with data-aware attention gate.

Math (per reference):
    src,dst = edges + self-loops; deg = bincount(dst); dinv = rsqrt(deg)
    norm = dinv[src]*dinv[dst]
    h1 = relu(segsum(norm * (x@W1)[src], dst) + b1)
    h2 = relu(segsum(norm * (h1@W2)[src], dst) + b2)
    out = h2 * sigmoid(h2@attn_w + attn_b)

v3 design (8 NeuronCores, dst-sharded, 128-slot windows):
  - Tables are pure bf16, node-major, pair-packed: gather element = two
    consecutive slots' rows = 256B.  Both layers share ONE edge plan:
    idx = s_pos>>1 (int16-safe), parity h = s_pos&1 selects the element
    half.  Layer-1 rows are 64 bf16 feats; layer-2 rows [32 feats|32 z].
  - Gathers are split into <=2048-element pieces across the 4 SWDGE
    queues so the 128-entry descriptor FIFO never starves the SDMA
    engines (measured 2.5x drain-rate difference).
  - Transposed aggregation: matmul(lhsT=gathered-chunk[128e x used],
    rhs=S[128e x 128 slots]) accumulates feat-major [used, 128] PSUM per
    window; no per-window PE transposes.
  - S one-hot matrices built ONE wide is_equal per gather-group via
    stride-0 broadcast APs (dval col vs iota row).
  - Self-loops excluded from edge lists; handled as one PE matmul per
    window: lhsT = own shard rows (SBUF-resident), rhs = identity.
  - All per-partition scalings ride scalar-engine ACT scale/bias.
"""

import sys

import numpy as np

_CONC = "/opt/trn_rl_repo"
if _CONC not in sys.path:
    sys.path.insert(0, _CONC)

# ---------------------------------------------------------------------------
# configuration
# ---------------------------------------------------------------------------


class Cfg:
    def __init__(self, N=50000, DIN=128, DH=64, DOUT=32, NC=8, WPC=49, WPG=5,
                 GSPLIT=2048):
        self.N, self.DIN, self.DH, self.DOUT = N, DIN, DH, DOUT
        self.NC, self.WPC, self.WPG, self.GSPLIT = NC, WPC, WPG, GSPLIT
        # variable-size window groups: WPG windows each, remainder in last
        self.groups = []
        w0 = 0
        while w0 < WPC:
            nw = min(WPG, WPC - w0)
            self.groups.append((w0, nw))
            w0 += nw
        self.G = len(self.groups)
        self.NPC = WPC * 128            # slots per core
        self.TOT = NC * self.NPC
        self.PAIRS = self.TOT // 2
        assert self.PAIRS <= 32768      # int16 gather indices
        assert self.N <= self.TOT


FULL = Cfg()

# ---------------------------------------------------------------------------
# host-side graph prep (structure only)
# ---------------------------------------------------------------------------


def _assign_slots(load, cfg):
    """LPT-deal nodes into NC*WPC bins of <=128 slots, balancing `load`."""
    import heapq

    nbins = cfg.NC * cfg.WPC
    order = np.argsort(-load, kind="stable")
    heap = [(0, b) for b in range(nbins)]
    heapq.heapify(heap)
    count = np.zeros(nbins, np.int64)
    pos = np.empty(cfg.N, np.int64)
    for n in order:
        l, b = heapq.heappop(heap)
        pos[n] = b * 128 + count[b]
        count[b] += 1
        if count[b] < 128:
            heapq.heappush(heap, (l + int(load[n]), b))
    return pos


def prep(x, edge_index, cfg):
    N, NC, WPC, WPG, G = cfg.N, cfg.NC, cfg.WPC, cfg.WPG, cfg.G
    NPC, DIN = cfg.NPC, cfg.DIN

    src = edge_index[0].astype(np.int64)
    dst = edge_index[1].astype(np.int64)
    deg_in = np.bincount(dst, minlength=N).astype(np.int64)
    deg = (deg_in + 1).astype(np.float32)          # + self-loop
    dinv = (1.0 / np.sqrt(np.maximum(deg, 1e-12))).astype(np.float32)

    pos = _assign_slots(deg_in, cfg)

    node_of = np.full(cfg.TOT, -1, np.int64)
    node_of[pos] = np.arange(N)
    dinv_slot = np.ones(cfg.TOT, np.float32)
    dinv_slot[pos] = dinv

    # edge records (no self-loops)
    s_pos = pos[src]
    d_pos = pos[dst]
    c_e = d_pos // NPC
    w_e = (d_pos % NPC) // 128
    dval_e = (d_pos % 128 + 2).astype(np.float32)
    h_e = s_pos & 1
    gidx_e = s_pos >> 1

    key_all = (c_e * WPC + w_e) * 2 + h_e
    order_e = np.argsort(key_all, kind="stable")
    ks = key_all[order_e]
    bounds = np.searchsorted(ks, np.arange(NC * WPC * 2 + 1))
    buckets = {}
    for key in range(NC * WPC * 2):
        lo, hi = bounds[key], bounds[key + 1]
        if hi > lo:
            buckets[key] = order_e[lo:hi]

    tgt = np.zeros((WPC, 2), np.int64)
    for w in range(WPC):
        for h in range(2):
            mx = max(len(buckets.get((c * WPC + w) * 2 + h, ()))
                     for c in range(NC))
            tgt[w, h] = int(np.ceil(max(mx, 1) / 128) * 128)

    seglen = np.zeros((G, 2), np.int64)
    for g, (w0, nw) in enumerate(cfg.groups):
        for h in range(2):
            seglen[g, h] = tgt[w0:w0 + nw, h].sum()

    idx_cols = int(sum(int(seglen[g, h]) // 16
                       for g in range(G) for h in range(2)))
    chunk_tot = int(sum(int(seglen[g, h]) // 128
                        for g in range(G) for h in range(2)))
    ioff, coff = {}, {}
    io = co = 0
    for g in range(G):
        for h in range(2):
            ioff[(g, h)] = io
            coff[(g, h)] = co
            io += int(seglen[g, h]) // 16
            co += int(seglen[g, h]) // 128
    wcol = np.zeros((WPC, 2), np.int64)
    for g, (w0, nw) in enumerate(cfg.groups):
        for h in range(2):
            c0 = coff[(g, h)]
            for wl in range(nw):
                w = w0 + wl
                wcol[w, h] = c0
                c0 += int(tgt[w, h]) // 128

    import ml_dtypes
    bf16 = ml_dtypes.bfloat16
    ix_all = np.zeros((NC, 128, idx_cols), np.int16)
    dvl_all = np.full((NC, 128, chunk_tot), -1.0, np.float32)
    for c in range(NC):
        for g, (w0, nw) in enumerate(cfg.groups):
            for h in range(2):
                n = int(seglen[g, h])
                gi = np.zeros(n, np.int64)
                dv = np.full(n, -1.0, np.float32)
                p = 0
                for wl in range(nw):
                    w = w0 + wl
                    es = buckets.get((c * WPC + w) * 2 + h, ())
                    ne = len(es)
                    if ne:
                        es = np.asarray(es)[np.argsort(gidx_e[es],
                                                       kind="stable")]
                    gi[p:p + ne] = gidx_e[es]
                    dv[p:p + ne] = dval_e[es]
                    p += int(tgt[w, h])
                wrapped = gi.reshape(n // 16, 16).T.astype(np.int16)
                ix_all[c, :, ioff[(g, h)]:ioff[(g, h)] + n // 16] = np.tile(
                    wrapped, (8, 1))
                dvl_all[c, :, coff[(g, h)]:coff[(g, h)] + n // 128] = (
                    dv.reshape(n // 128, 128).T)

    X_all = np.zeros((cfg.TOT, DIN), np.float32)
    X_all[pos] = np.asarray(x, np.float32)
    xT_sh = np.zeros((NC, DIN, NPC), bf16)
    dv128 = np.zeros((NC, 128, WPC), np.float32)
    dvrep = np.zeros((NC, 64, NPC), np.float32)
    for c in range(NC):
        sl = slice(c * NPC, (c + 1) * NPC)
        xT_sh[c] = X_all[sl].T.astype(bf16)
        ds = dinv_slot[sl]
        dv128[c] = ds.reshape(WPC, 128).T
        dvrep[c] = np.tile(ds, (64, 1))

    plan = dict(tgt=tgt, seglen=seglen, ioff=ioff, coff=coff, wcol=wcol,
                idx_cols=idx_cols, chunk_tot=chunk_tot)
    host = dict(xT_sh=xT_sh, dv128=dv128, dvrep=dvrep, ix_all=ix_all,
                dvl_all=dvl_all.astype(bf16), pos=pos)
    return plan, host


# ---------------------------------------------------------------------------
# device kernel
# ---------------------------------------------------------------------------


def build(cfg, plan):
    import concourse.bass as bass  # noqa: F401
    import concourse.mybir as mybir
    import concourse.tile as tile
    from concourse import bacc

    NC, WPC, WPG, G = cfg.NC, cfg.WPC, cfg.WPG, cfg.G
    NPC, TOT = cfg.NPC, cfg.TOT
    DH, DOUT = cfg.DH, cfg.DOUT
    f32 = mybir.dt.float32
    bf16 = mybir.dt.bfloat16
    AF = mybir.ActivationFunctionType
    tgt, seglen = plan["tgt"], plan["seglen"]
    ioff, coff, wcol = plan["ioff"], plan["coff"], plan["wcol"]

    nc = bacc.Bacc(
        "TRN2", target_bir_lowering=False, debug=False,
        num_devices=NC, num_swdge_queues=4,
    )

    xT_d = nc.dram_tensor("xT", [128, NPC], bf16, kind="ExternalInput")
    w1_d = nc.dram_tensor("w1", [128, DH], bf16, kind="ExternalInput")
    w2_d = nc.dram_tensor("w2", [DH, DOUT], bf16, kind="ExternalInput")
    dv128_d = nc.dram_tensor("dv128", [128, WPC], f32, kind="ExternalInput")
    dvrep_d = nc.dram_tensor("dvrep", [64, NPC], f32, kind="ExternalInput")
    id_d = nc.dram_tensor("ident", [128, 128], bf16, kind="ExternalInput")
    b1_d = nc.dram_tensor("b1c", [DH, 1], f32, kind="ExternalInput")
    b2_d = nc.dram_tensor("b2c", [DOUT, 1], f32, kind="ExternalInput")
    idaw_d = nc.dram_tensor("idaw", [DOUT, DOUT + 1], bf16,
                            kind="ExternalInput")
    ab_d = nc.dram_tensor("abc", [128, 1], f32, kind="ExternalInput")
    gi_d = nc.dram_tensor("gi128", [128, 128], bf16, kind="ExternalInput")
    ix_d = nc.dram_tensor("ix", [128, plan["idx_cols"]], mybir.dt.int16,
                          kind="ExternalInput")
    dvl_d = nc.dram_tensor("dvl", [128, plan["chunk_tot"]], bf16,
                           kind="ExternalInput")
    out_d = nc.dram_tensor("out_sh", [NPC, DOUT], f32, kind="ExternalOutput")

    rg = [list(range(NC))]
    qctr = [0]

    with tile.TileContext(nc) as tc:
        with tc.tile_pool(name="const", bufs=1) as cpool:
            def load(dram, shape, dt=f32):
                t = cpool.tile(shape, dt, tag=dram.name, name=dram.name + "_s")
                nc.sync.dma_start(t[:], dram.ap())
                return t

            w1_s = load(w1_d, [128, DH], bf16)
            w2_s = load(w2_d, [DH, DOUT], bf16)
            dv128_s = load(dv128_d, [128, WPC])
            dvrep_s = load(dvrep_d, [64, NPC])
            id_s = load(id_d, [128, 128], bf16)
            b1_s = load(b1_d, [DH, 1])
            b2_s = load(b2_d, [DOUT, 1])
            idaw_s = load(idaw_d, [DOUT, DOUT + 1], bf16)
            ab_s = load(ab_d, [128, 1])
            gi_s = load(gi_d, [128, 128], bf16)
            ix_s = load(ix_d, [128, plan["idx_cols"]], mybir.dt.int16)
            dvl_s = load(dvl_d, [128, plan["chunk_tot"]], bf16)

            t1sb = cpool.tile([128, WPC * 64], bf16, tag="t1sb", name="t1sb")
            t2sb = cpool.tile([128, WPC * 64], bf16, tag="t2sb", name="t2sb")
            h2at = cpool.tile([128, WPC * 33], f32, tag="h2at", name="h2at")
            nc.vector.memset(t2sb[:], 0.0)

            with tc.tile_pool(name="dram", bufs=1, space="DRAM") as dpool:
                t1_shard = dpool.tile([NPC, DH], bf16, tag="t1s", name="t1s")
                t1_full = dpool.tile([TOT, DH], bf16, tag="t1f", name="t1f",
                                     addr_space="Shared")
                t2_shard = dpool.tile([NPC, DH], bf16, tag="t2s", name="t2s")
                t2_full = dpool.tile([TOT, DH], bf16, tag="t2f", name="t2f",
                                     addr_space="Shared")

                # ---- phase 1: t1 = dinv .* (x @ W1), node-major bf16
                with (
                    tc.tile_pool(name="ph1", bufs=1) as ph1,
                    tc.tile_pool(name="ph1ps", bufs=4, space="PSUM") as pps,
                ):
                    xts = ph1.tile([128, NPC], bf16, tag="xts", name="xts")
                    nc.sync.dma_start(xts[:], xT_d.ap())
                    for b in range(WPC):
                        ps = pps.tile([128, DH], f32, tag="p1", name="p1")
                        nc.tensor.matmul(ps[:],
                                         lhsT=xts[:, b * 128:(b + 1) * 128],
                                         rhs=w1_s[:], start=True, stop=True)
                        nc.scalar.activation(
                            t1sb[:, b * 64:(b + 1) * 64], ps[:],
                            func=AF.Copy, scale=dv128_s[:, b:b + 1])
                    nc.sync.dma_start(
                        t1_shard.rearrange("(w p) f -> p w f", p=128),
                        t1sb[:].rearrange("p (w f) -> p w f", f=64))

                nc.gpsimd.collective_compute(
                    "AllGather", mybir.AluOpType.bypass, replica_groups=rg,
                    ins=[t1_shard[:]], outs=[t1_full[:]],
                )

                # ---- shared aggregation loop
                def aggregate(full, used, sl_lhs, flush_fn):
                    fv = full.rearrange("(a b) d -> a (b d)", b=2)
                    with (
                        tc.tile_pool(name="gp", bufs=3) as gp,
                        tc.tile_pool(name="sp", bufs=2) as sp,
                        tc.tile_pool(name="aps", bufs=4, space="PSUM") as aps,
                        tc.tile_pool(name="fsb", bufs=3) as fsb,
                        tc.tile_pool(name="fps", bufs=2, space="PSUM") as fps,
                    ):
                        PC = cfg.GSPLIT // 128   # chunks per gather piece
                        for g, (w0, nw) in enumerate(cfg.groups):
                            gts, Ss = {}, {}
                            for h in range(2):
                                n = int(seglen[g, h])
                                nch = n // 128
                                io = ioff[(g, h)]
                                pieces = []
                                o0 = 0
                                j = 0
                                while o0 < n:
                                    nn = min(cfg.GSPLIT, n - o0)
                                    pt = gp.tile([128, nn], bf16,
                                                 tag=f"g{h}p{j}",
                                                 name=f"gt{h}p{j}")
                                    nc.gpsimd.dma_gather(
                                        out_ap=pt[:, 0:nn].rearrange(
                                            "p (c d) -> p c d", d=128),
                                        in_ap=fv,
                                        idxs_ap=ix_s[:, io + o0 // 16:
                                                     io + (o0 + nn) // 16],
                                        num_idxs=nn, num_idxs_reg=nn,
                                        elem_size=128, elem_step=128,
                                        queue_num=qctr[0] % 4,
                                        single_packet=False,
                                    )
                                    qctr[0] += 1
                                    pieces.append(pt)
                                    o0 += nn
                                    j += 1
                                c0 = coff[(g, h)]
                                S = sp.tile([128, nch * 128], bf16,
                                            tag=f"S{h}", name=f"S{h}")
                                nc.vector.tensor_tensor(
                                    out=S[:].rearrange(
                                        "p (c j) -> p c j", j=128),
                                    in0=dvl_s[:, c0:c0 + nch].unsqueeze(2)
                                    .broadcast_to((128, nch, 128)),
                                    in1=gi_s[:].unsqueeze(1)
                                    .broadcast_to((128, nch, 128)),
                                    op=mybir.AluOpType.is_equal,
                                )
                                gts[h], Ss[h] = pieces, S
                            for wl in range(nw):
                                w = w0 + wl
                                ps = aps.tile([used, 128], f32, tag="agg",
                                              name="agg")
                                nc.tensor.matmul(
                                    ps[:], lhsT=sl_lhs(w), rhs=id_s[:],
                                    start=True, stop=False)
                                chunks = (
                                    [(0, k)
                                     for k in range(int(tgt[w, 0]) // 128)]
                                    + [(1, k)
                                       for k in range(int(tgt[w, 1]) // 128)])
                                for j, (h, k) in enumerate(chunks):
                                    kk = int(wcol[w, h] - coff[(g, h)]) + k
                                    pt = gts[h][kk // PC]
                                    base = (kk % PC) * 128 + h * 64
                                    nc.tensor.matmul(
                                        ps[:],
                                        lhsT=pt[:, base:base + used],
                                        rhs=Ss[h][:, kk * 128:(kk + 1) * 128],
                                        start=False,
                                        stop=(j == len(chunks) - 1))
                                flush_fn(w, ps, fsb, fps)

                # ---- layer-1 flush
                def flush1(w, ps, fsb, fps):
                    a = fsb.tile([64, 128], f32, tag="a", name="a")
                    nc.vector.tensor_tensor(
                        out=a[:], in0=ps[:],
                        in1=dvrep_s[:, w * 128:(w + 1) * 128],
                        op=mybir.AluOpType.mult)
                    hT = fsb.tile([64, 128], bf16, tag="hT", name="hT")
                    nc.scalar.activation(hT[:], a[:], func=AF.Relu,
                                         bias=b1_s[:, 0:1])
                    t2ps = fps.tile([128, DOUT], f32, tag="t2ps", name="t2ps")
                    nc.tensor.matmul(t2ps[:], lhsT=hT[:], rhs=w2_s[:],
                                     start=True, stop=True)
                    nc.scalar.activation(
                        t2sb[:, w * 64:w * 64 + DOUT], t2ps[:],
                        func=AF.Copy, scale=dv128_s[:, w:w + 1])

                def sl1(w):
                    return t1sb[:, w * 64:(w + 1) * 64]

                aggregate(t1_full[:], DH, sl1, flush1)
                nc.sync.dma_start(
                    t2_shard.rearrange("(w p) f -> p w f", p=128),
                    t2sb[:].rearrange("p (w f) -> p w f", f=64))

                nc.gpsimd.collective_compute(
                    "AllGather", mybir.AluOpType.bypass, replica_groups=rg,
                    ins=[t2_shard[:]], outs=[t2_full[:]],
                )

                # ---- layer-2 flush
                def flush2(w, ps, fsb, fps):
                    a2 = fsb.tile([DOUT, 128], f32, tag="a2", name="a2")
                    nc.vector.tensor_tensor(
                        out=a2[:], in0=ps[:],
                        in1=dvrep_s[:DOUT, w * 128:(w + 1) * 128],
                        op=mybir.AluOpType.mult)
                    h2T = fsb.tile([DOUT, 128], bf16, tag="h2T", name="h2T")
                    nc.scalar.activation(h2T[:], a2[:], func=AF.Relu,
                                         bias=b2_s[:, 0:1])
                    gps = fps.tile([128, DOUT + 1], f32, tag="gps", name="gps")
                    nc.tensor.matmul(gps[:], lhsT=h2T[:], rhs=idaw_s[:],
                                     start=True, stop=True)
                    nc.scalar.activation(
                        h2at[:, w * 33:(w + 1) * 33], gps[:], func=AF.Copy)

                def sl2(w):
                    return t2sb[:, w * 64:w * 64 + DOUT]

                aggregate(t2_full[:], DOUT, sl2, flush2)

                # ---- attention gate tail
                with tc.tile_pool(name="tail", bufs=1) as tp:
                    atall = tp.tile([128, WPC], f32, tag="atall", name="atall")
                    nc.scalar.activation(
                        atall[:],
                        h2at[:].rearrange("p (w q) -> p w q", q=33)[:, :, 32],
                        func=AF.Sigmoid, bias=ab_s[:, 0:1])
                    oall = tp.tile([128, WPC * DOUT], f32, tag="oall",
                                   name="oall")
                    nc.vector.tensor_tensor(
                        out=oall[:].rearrange("p (w f) -> p w f", f=DOUT),
                        in0=h2at[:].rearrange(
                            "p (w q) -> p w q", q=33)[:, :, 0:DOUT],
                        in1=atall[:].unsqueeze(2)
                        .broadcast_to((128, WPC, DOUT)),
                        op=mybir.AluOpType.mult)
                    nc.sync.dma_start(
                        out_d.ap().rearrange("(w p) f -> p w f", p=128),
                        oall[:].rearrange("p (w f) -> p w f", f=DOUT))

    nc.compile()
    return nc


# ---------------------------------------------------------------------------
# entry point
# ---------------------------------------------------------------------------


def _make_in_maps(cfg, host, W1, b1, W2, b2, attn_w, attn_b):
    import ml_dtypes
    bf16 = ml_dtypes.bfloat16
    giota = np.tile(np.arange(2, 130, dtype=np.float32),
                    (128, 1)).astype(bf16)
    idaw = np.concatenate(
        [np.eye(cfg.DOUT, dtype=np.float32),
         np.asarray(attn_w, np.float32).reshape(cfg.DOUT, 1)],
        axis=1).astype(bf16)
    in_maps = []
    for c in range(cfg.NC):
        in_maps.append({
            "xT": host["xT_sh"][c],
            "w1": np.asarray(W1, np.float32).astype(bf16),
            "w2": np.asarray(W2, np.float32).astype(bf16),
            "dv128": host["dv128"][c],
            "dvrep": host["dvrep"][c],
            "ident": np.eye(128, dtype=np.float32).astype(bf16),
            "b1c": np.asarray(b1, np.float32).reshape(cfg.DH, 1),
            "b2c": np.asarray(b2, np.float32).reshape(cfg.DOUT, 1),
            "idaw": idaw,
            "abc": np.full((128, 1),
                           np.asarray(attn_b, np.float32).reshape(-1)[0],
                           np.float32),
            "gi128": giota,
            "ix": host["ix_all"][c],
            "dvl": host["dvl_all"][c],
        })
    return in_maps


def run(x, edge_index, W1, b1, W2, b2, attn_w, attn_b, cfg=None,
        backend="hw", trace=False):
    cfg = cfg or FULL
    plan, host = prep(x, edge_index, cfg)
    nc = build(cfg, plan)
    in_maps = _make_in_maps(cfg, host, W1, b1, W2, b2, attn_w, attn_b)

    if backend == "sim":
        from concourse.bass_interp import MultiCoreSim
        sim = MultiCoreSim(nc, num_cores=cfg.NC, trace=False)
        for c, core in enumerate(sim.cores.values()):
            for name, arr in in_maps[c].items():
                core.tensor(name)[:] = arr
        sim.simulate()
        outs = [core.tensor("out_sh").copy() for core in sim.cores.values()]
        exec_ns = None
    else:
        from concourse import bass_utils
        from concourse.bass_interp import get_hw_module
        old = nc.m
        nc.m = get_hw_module(nc.m)
        try:
            res = bass_utils.run_bass_kernel_spmd(
                nc, in_maps, core_ids=list(range(cfg.NC)), trace=trace)
        finally:
            nc.m = old
        outs = [res.results[c]["out_sh"] for c in range(cfg.NC)]
        exec_ns = res.exec_time_ns

    full = np.concatenate(outs, axis=0)   # [TOT, DOUT] in slot order
    out = full[host["pos"]]               # unpermute -> [N, DOUT]
    return np.ascontiguousarray(out), exec_ns


def kernel(x, edge_index, W1, b1, W2, b2, attn_w, attn_b):
    out, _ = run(x, edge_index, W1, b1, W2, b2, attn_w, attn_b,
                 cfg=FULL, backend="hw", trace=False)
    return out
